# revision 38
# baseline (speedup 1.0000x reference)
"""Trainium2 Bass kernel for a 6-layer transformer encoder (B=4, S=1024,
d_model=1024, 16 heads, d_ff=4096).

Sharding: token-parallel across 8 cores (B*S = 4096 tokens -> 512/core; each
core owns half of one batch element's sequence).  Per layer, pair-wise
AllGathers of K^T (bf16, 2 chunks) and V (fp8, vaug layout) let each core
rebuild K/V for its full batch element.

v5: the LayerNorm fold now pre-scales the residual once per layer
(rbn = rb * rstd, 8 cheap DVE bf16 muls) instead of applying the per-token
rstd at every PSUM drain.  All projection drains become single Scalar-engine
activations (Identity/Relu/Copy with per-partition bias), eliminating the
serialized 7.5us GpSimd tensor_scalar chain that dominated v4's critical
path.  The LN affine (g, be) is folded into the next projection: be rides a
K=1 ones-outer-product PSUM seed, g rides the drain's scalar_tensor_tensor.
Reciprocals use the ~5x faster approx-NR custom DVE op.  LN2's x-hat is kept
unscaled in the residual stream; the final layer applies g2/be2 on the way
out.
"""

import sys
import os

for _p in ("/opt/trn_rl_repo", "/root/.axon_site/_ro/trn_rl_repo"):
    if os.path.isdir(_p) and _p not in sys.path:
        sys.path.insert(0, _p)

import numpy as np
import ml_dtypes

import concourse.bass as bass
import concourse.mybir as mybir
import concourse.tile as tile
from concourse.bass_utils import run_bass_kernel_spmd
from concourse.masks import make_identity

VOCAB, D, H, DFF, L = 32000, 1024, 16, 4096, 6
B, S = 4, 1024
DK = D // H              # 64
NCORES = 8
TOK = (B * S) // NCORES  # 512 tokens per core
KT = D // 128            # 8
FT = DFF // 128          # 32
EPS = 1e-5
VS = 16.0                # fp8 V pre-scale (ones col carries the same scale)

F32 = mybir.dt.float32
BF16 = mybir.dt.bfloat16
FP8 = mybir.dt.float8e4
I32 = mybir.dt.int32
AF = mybir.ActivationFunctionType
OP = mybir.AluOpType
DR = mybir.MatmulPerfMode.DoubleRow

_NC = None
DEBUG = False

PAIRS = [[2 * i, 2 * i + 1] for i in range(NCORES // 2)]


def _build_nc():
    nc = bass.Bass("TRN2", target_bir_lowering=False, debug=False, num_devices=NCORES)

    emb = nc.dram_tensor("emb", [VOCAB, D], F32, kind="ExternalInput")
    src = nc.dram_tensor("src", [TOK, 1], I32, kind="ExternalInput")
    peT = nc.dram_tensor("peT", [D, TOK], F32, kind="ExternalInput")
    maskb = nc.dram_tensor("maskb", [128, KT], F32, kind="ExternalInput")
    koidx = nc.dram_tensor("koidx", [512, 1], I32, kind="ExternalInput")
    voidx = nc.dram_tensor("voidx", [TOK, 1], I32, kind="ExternalInput")
    # projection weights pre-tiled host-side to [L, chunk, 128, KT, cols]
    # so every weight DMA is a fully contiguous read
    wq = nc.dram_tensor("wq", [L, 2, 128, KT, 512], BF16, kind="ExternalInput")
    wk = nc.dram_tensor("wk", [L, 2, 128, KT, 512], BF16, kind="ExternalInput")
    wv = nc.dram_tensor("wv", [L, 2, 128, KT, 512], BF16, kind="ExternalInput")
    wo = nc.dram_tensor("wo", [L, 2, 128, KT, 512], BF16, kind="ExternalInput")
    w1 = nc.dram_tensor("w1", [L, 8, 128, KT, 512], BF16, kind="ExternalInput")
    w2 = nc.dram_tensor("w2", [L, DFF, D], BF16, kind="ExternalInput")
    bqT = nc.dram_tensor("bqT", [L, 128, KT], F32, kind="ExternalInput")
    bkT = nc.dram_tensor("bkT", [L, 128, KT], F32, kind="ExternalInput")
    b1T = nc.dram_tensor("b1T", [L, 128, FT], F32, kind="ExternalInput")
    g1T = nc.dram_tensor("g1T", [L, 128, KT], F32, kind="ExternalInput")
    g2T = nc.dram_tensor("g2T", [L, 128, KT], F32, kind="ExternalInput")
    be2T = nc.dram_tensor("be2T", [L, 128, KT], F32, kind="ExternalInput")
    # K=1 PSUM-seed rows: fc2 gets b2+be1, O-proj gets bo_eff(+be2[l-1])
    b2r = nc.dram_tensor("b2r", [L, 1, D], BF16, kind="ExternalInput")
    bor = nc.dram_tensor("bor", [L, 1, D], BF16, kind="ExternalInput")
    xout = nc.dram_tensor("xout", [D, TOK], F32, kind="ExternalOutput")
    dbg = {}
    if DEBUG:
        for nm, shp, dt in [
            ("dbg_x0", [128, KT, TOK], F32),     # post-embedding x
            ("dbg_ktl", [128, KT, TOK], BF16),   # K proj (own half)
            ("dbg_qt", [128, KT, TOK], BF16),    # Q proj
            ("dbg_attn", [128, KT, TOK], BF16),  # softmax @ V
            ("dbg_r1", [128, KT, TOK], F32),     # post O-drain residual
            ("dbg_ht", [128, FT, TOK], BF16),    # fc1 relu out
            ("dbg_r2", [128, KT, TOK], F32),     # post fc2 residual
            ("dbg_xh2", [128, KT, TOK], F32),    # xhat2 end of layer 0
        ]:
            dbg[nm] = nc.dram_tensor(nm, shp, dt, kind="ExternalOutput")

    from contextlib import ExitStack
    with tile.TileContext(nc) as tc:
        with ExitStack() as _es:
            def _pool(**kw):
                return _es.enter_context(tc.tile_pool(**kw))
            cpool = _pool(name="cpool", bufs=1)
            wp = _pool(name="wp", bufs=2)        # QKVO/fc1 weight chunks
            w2p = _pool(name="w2p", bufs=3)      # fc2 weight row-blocks
            pbig = _pool(name="pbig", bufs=1)    # ht
            ppe = _pool(name="ppe", bufs=1)      # peT (embedding only)
            px = _pool(name="px", bufs=1)        # f32 residual (in-place)
            pxb = _pool(name="pxb", bufs=2)      # bf16 pre-norm rb ping-pong
            pxn = _pool(name="pxn", bufs=2)      # bf16 rstd-scaled rbn
            pat = _pool(name="pat", bufs=1)      # ktl/kto/qt/attn
            pexp = _pool(name="pexp", bufs=2)    # exps per head-pair
            p2 = _pool(name="p2", bufs=2)        # transients (sq/stages)
            prs = _pool(name="prs", bufs=2)      # rstd rows
            bp = _pool(name="bp", bufs=8)        # bias rows
            psd = _pool(name="psd", bufs=2)      # K=1 PSUM-seed rows
            psc = _pool(name="psc", bufs=2, space="PSUM")   # 2-bank tiles
            ppv = _pool(name="ppv", bufs=2, space="PSUM")   # 1-bank tiles
            ppj = _pool(name="ppj", bufs=2, space="PSUM")   # proj psums
            dram = _pool(name="dram", bufs=2, space="DRAM")
            _uid = [0]

            def _nm(tag):
                _uid[0] += 1
                return f"{tag}_{_uid[0]}"

            ident = cpool.tile([128, 128], BF16, tag="ident", name=_nm("ident"))
            make_identity(nc, ident[:])
            onesk = cpool.tile([128, 128], BF16, tag="onesk", name=_nm("onesk"))
            nc.vector.memset(onesk[:], 1.0 / D)
            ones1t = cpool.tile([1, TOK], BF16, tag="ones1t", name=_nm("ones1t"))
            nc.vector.memset(ones1t[:], 1.0)
            maskb_sb = cpool.tile([128, KT], F32, tag="maskb", name=_nm("maskb"))
            nc.sync.dma_start(maskb_sb[:], maskb[:])
            eps_sb = cpool.tile([128, 1], F32, tag="eps", name=_nm("eps"))
            nc.vector.memset(eps_sb[:], EPS)
            # vaug: [tok-part, kt, head*65] fp8; col 64 of each head block is
            # the constant VS used to accumulate the softmax denominator.
            vaug = cpool.tile([128, KT, H * 65], FP8, tag="vaug", name=_nm("vaug"))
            nc.vector.memset(
                vaug[:].rearrange("p t (h x) -> p t h x", x=65)[:, :, :, 64:65], VS)

            def pj():
                return ppj.tile([128, 512], F32, tag="pj", name=_nm("pj"))

            def load_bias8(t, l):
                b = bp.tile([128, KT], F32, tag="bias8", name=_nm("bias8"))
                nc.sync.dma_start(b[:], t[l])
                return b

            def load_seed(t, l):
                s = psd.tile([1, D], BF16, tag="seedrow", name=_nm("seed"))
                nc.sync.dma_start(s[:], t[l])
                return s

            # ---------------- embedding ----------------
            peT_sb = ppe.tile([128, KT, TOK], F32, tag="peT", name=_nm("peT"))
            nc.sync.dma_start(peT_sb[:], peT.rearrange("(t p) n -> p t n", p=128))
            x_cur = px.tile([128, KT, TOK], F32, tag="x", name=_nm("x"))
            for blk in range(TOK // 128):
                idx_t = p2.tile([128, 1], I32, tag="idx", name=_nm("idx"))
                nc.sync.dma_start(idx_t[:], src[blk * 128:(blk + 1) * 128, :])
                gat = p2.tile([128, D], F32, tag="bfs", name=_nm("gat"))
                nc.gpsimd.indirect_dma_start(
                    out=gat[:], out_offset=None, in_=emb[:],
                    in_offset=bass.IndirectOffsetOnAxis(ap=idx_t[:, :1], axis=0),
                )
                gatb = p2.tile([128, D], BF16, tag="lns", name=_nm("gatb"))
                nc.scalar.activation(gatb[:], gat[:], AF.Copy)
                for kt in range(KT):
                    tp = ppj.tile([128, 512], BF16, tag="pj", name=_nm("tp"))
                    nc.tensor.transpose(tp[:, :128], gatb[:, kt * 128:(kt + 1) * 128],
                                        ident[:])
                    nc.vector.scalar_tensor_tensor(
                        out=x_cur[:, kt, blk * 128:(blk + 1) * 128],
                        in0=tp[:, :128], scalar=32.0,
                        in1=peT_sb[:, kt, blk * 128:(blk + 1) * 128],
                        op0=OP.mult, op1=OP.add,
                    )
            rbn_cur = pxn.tile([128, KT, TOK], BF16, tag="xn", name=_nm("xn0"))
            for kk in range(KT):
                nc.scalar.activation(rbn_cur[:, kk, :], x_cur[:, kk, :], AF.Copy)
            if DEBUG:
                nc.sync.dma_start(dbg["dbg_x0"][:], x_cur[:])

            # ------------- layer norm pieces -------------
            def ln_begin():
                st = psc.tile([128, 2, 512], F32, tag="sc", name=_nm("lnst"))
                rb = pxb.tile([128, KT, TOK], BF16, tag="xb", name=_nm("rb"))
                return {"st": st, "rb": rb}

            def ln_stat(s, r, mg):
                nc.scalar.activation(s["rb"][:, mg, :], r[:, mg, :], AF.Copy)
                sq = p2.tile([128, TOK], BF16, tag="sq1", name=_nm("sq"))
                nc.vector.tensor_mul(sq[:], s["rb"][:, mg, :],
                                     s["rb"][:, mg, :])
                nc.tensor.matmul(s["st"][:, 0, :], onesk[:], s["rb"][:, mg, :],
                                 start=(mg == 0), stop=(mg == KT - 1))
                nc.tensor.matmul(s["st"][:, 1, :], onesk[:], sq[:],
                                 start=(mg == 0), stop=(mg == KT - 1))

            def ln_final(s):
                # rstd = sqrt(1/(var+eps)): the approx-reciprocal runs on the
                # raw variance so both the f32 and bf16 sqrt taps come straight
                # off one DVE chain (shorter than sqrt->recip->cast)
                msq = p2.tile([128, TOK], F32, tag="lns", name=_nm("msq"))
                nc.scalar.activation(msq[:], s["st"][:, 0, :], AF.Square)
                inv = p2.tile([128, TOK], F32, tag="lns", name=_nm("inv"))
                nc.vector.scalar_tensor_tensor(
                    out=inv[:], in0=s["st"][:, 1, :], scalar=eps_sb[:, 0:1],
                    in1=msq[:], op0=OP.add, op1=OP.subtract)
                nc.vector.reciprocal_approx_fast(inv[:], inv[:])
                rstd = prs.tile([128, TOK], F32, tag="rstd", name=_nm("rstd"))
                nc.scalar.activation(rstd[:], inv[:], AF.Sqrt)
                rs16 = prs.tile([128, TOK], BF16, tag="rs16", name=_nm("rs16"))
                nc.scalar.activation(rs16[:], inv[:], AF.Sqrt)
                s["rstd"] = rstd
                s["rs16"] = rs16

            def ln_xhat_kk(s, r, kk):
                # in-place normalize of the residual stream (r -> x-hat,
                # WITHOUT the g/be affine - that is folded downstream)
                nc.vector.tensor_sub(r[:, kk, :], r[:, kk, :], s["st"][:, 0, :])
                nc.vector.tensor_mul(r[:, kk, :], r[:, kk, :], s["rstd"][:])

            # ---------------- layers ----------------
            for l in range(L):
                bk_sb = load_bias8(bkT, l)
                bq_sb = load_bias8(bqT, l)
                bor_sb = load_seed(bor, l)
                rbn_in = rbn_cur        # rstd-scaled residual (or embedding)

                # --- K projection + chunked pair-AllGather of K^T ---
                # weight loads ride the ACT queue (nc.scalar) one phase ahead
                # of their consumers so the PE never waits on LDWEIGHTS input
                def wload(t, src):
                    w = wp.tile([128, KT, 512], BF16, tag="wproj", name=_nm(t))
                    nc.scalar.dma_start(w[:], src)
                    return w

                wkh_t = [wload("wk", wk[l, h]) for h in range(2)]
                wvh_t = [None, None]
                wq_tiles = {}
                ktl = pat.tile([128, KT, TOK], BF16, tag="ktl", name=_nm("ktl"))
                kag_in = dram.tile([D, TOK], BF16, tag="kag_in", name=_nm("kag_in"))
                kag_out = [
                    dram.tile([2 * 512, TOK], BF16, tag=f"kag_out{c}",
                              name=_nm("kag_out")) for c in range(2)]
                kto = pat.tile([128, KT, TOK], BF16, tag="kto", name=_nm("kto"))
                for half in range(2):
                    wkh = wkh_t[half]
                    for m in range(4):
                        mg = half * 4 + m
                        pt = pj()
                        for kk in range(KT):
                            nc.tensor.matmul(
                                pt[:], wkh[:, kk, m * 128:(m + 1) * 128],
                                rbn_in[:, kk, :],
                                start=(kk == 0), stop=(kk == KT - 1))
                        nc.scalar.activation(ktl[:, mg, :], pt[:], AF.Identity,
                                             bias=bk_sb[:, mg:mg + 1])
                        nc.sync.dma_start(
                            kag_in[mg * 128:(mg + 1) * 128, :], ktl[:, mg, :])
                    nc.gpsimd.collective_compute(
                        "AllGather", OP.bypass,
                        ins=[kag_in[half * 512:(half + 1) * 512, :]],
                        outs=[kag_out[half][:]],
                        replica_groups=PAIRS,
                    )
                    wvh_t[half] = wload("wv", wv[l, half])
                    # partner-K gathers ride right behind this half's AG
                    for g in range(4 * half, 4 * half + 4):
                        kidx = bp.tile([128, 1], I32, tag="koidx",
                                       name=_nm("koidx"))
                        nc.sync.dma_start(
                            kidx[:], koidx[(g % 4) * 128:(g % 4) * 128 + 128, :])
                        nc.gpsimd.indirect_dma_start(
                            out=kto[:, g, :], out_offset=None,
                            in_=kag_out[g // 4][:],
                            in_offset=bass.IndirectOffsetOnAxis(
                                ap=kidx[:, :1], axis=0),
                        )
                if DEBUG and l == 0:
                    nc.sync.dma_start(dbg["dbg_ktl"][:], ktl[:])

                # --- V projection into vaug (own slots) + fp8 AllGather ---
                # mt-outer so each 128-token row block is complete (both head
                # halves) early; the AG is split in two so partner-V gathers
                # start at the V-phase midpoint instead of after it
                vag_in = dram.tile([TOK, H * 65], FP8, tag="vag_in", name=_nm("vag_in"))
                vag_out = [
                    dram.tile([2 * 256, H * 65], FP8, tag=f"vag_out{c}",
                              name=_nm("vag_out")) for c in range(2)]
                for mt in range(4):
                    for half in range(2):
                        wvh = wvh_t[half]
                        pt = pj()
                        for kk in range(KT):
                            nc.tensor.matmul(
                                pt[:], rbn_in[:, kk, mt * 128:(mt + 1) * 128],
                                wvh[:, kk, :],
                                start=(kk == 0), stop=(kk == KT - 1))
                        vdst = vaug[:, mt, :].rearrange(
                            "p (h x) -> p h x", x=65)[:, half * 8:(half + 1) * 8, 0:64]
                        vsrc = pt[:].rearrange("p (h c) -> p h c", c=64)
                        nc.scalar.activation(vdst, vsrc, AF.Copy, scale=VS)
                    nc.sync.dma_start(
                        vag_in[mt * 128:(mt + 1) * 128, :], vaug[:, mt, :])
                    if mt % 2 == 1:
                        hv = mt // 2
                        nc.gpsimd.collective_compute(
                            "AllGather", OP.bypass,
                            ins=[vag_in[hv * 256:(hv + 1) * 256, :]],
                            outs=[vag_out[hv][:]],
                            replica_groups=PAIRS,
                        )
                        if hv == 0:
                            wq_tiles[0] = wload("wq", wq[l, 0])
                        for mt2 in (2 * hv, 2 * hv + 1):
                            vidx = bp.tile([128, 1], I32, tag="voidx",
                                           name=_nm("voidx"))
                            nc.sync.dma_start(
                                vidx[:], voidx[mt2 * 128:(mt2 + 1) * 128, :])
                            nc.gpsimd.indirect_dma_start(
                                out=vaug[:, 4 + mt2, :], out_offset=None,
                                in_=vag_out[hv][:],
                                in_offset=bass.IndirectOffsetOnAxis(
                                    ap=vidx[:, :1], axis=0),
                            )
                wq_tiles[1] = wload("wq", wq[l, 1])

                # --- Q projection woven with attention at 2-MM granularity ---
                qt = pat.tile([128, KT, TOK], BF16, tag="qt", name=_nm("qt"))
                # per-m-tile attn tiles: O-proj's kk-th matmul depends only on
                # tile kk, so the tail heads' softmax chains overlap O-proj
                attn_t = [pat.tile([128, TOK], BF16, tag=f"attn{j}",
                                   name=_nm("attn")) for j in range(KT)]
                recips_d = dram.tile([H, TOK], F32, tag="recips_d", name=_nm("recd"))
                recips_r = dram.tile([H, TOK], F32, tag="recips_r", name=_nm("recr"))
                exps_tiles = {}

                def kt_lhs(kt, mj, prow):
                    if kt < 4:
                        return ktl[prow:prow + 64, mj, kt * 128:(kt + 1) * 128]
                    return kto[prow:prow + 64, mj, (kt - 4) * 128:(kt - 3) * 128]

                def s_unit(mj, kt):
                    # one kt-block of scores for BOTH heads of m-tile mj; a
                    # single exp instruction covers the pair (same mask col)
                    if kt == 0:
                        exps_tiles[mj] = pexp.tile([128, KT, 2, TOK], FP8,
                                                   tag="exps", name=_nm("ex"))
                    ex = exps_tiles[mj]
                    st = psc.tile([128, 2, 512], F32, tag="sc", name=_nm("sc"))
                    for j in range(2):
                        prow = j * 64
                        nc.tensor.matmul(
                            st[:, j, :], kt_lhs(kt, mj, prow),
                            qt[prow:prow + 64, mj, :],
                            start=True, stop=True)
                    nc.scalar.activation(
                        ex[:, kt, :, :], st[:, :, :], AF.Exp,
                        scale=DK ** -0.5, bias=maskb_sb[:, kt:kt + 1])

                av_state = {}
                av_pend = {}

                def av_flush(h):
                    # the attn mul runs one head-slot after its rbc broadcast
                    # was issued: the DMA lands during the gap, so the mul
                    # never head-of-line-blocks the DVE queue
                    mj, pavs, rbc = av_pend.pop(h)
                    if h % 2 == 0:
                        nc.vector.tensor_mul(
                            attn_t[mj][0:64, :], pavs[0:64, :], rbc[:])
                    else:
                        stg = p2.tile([64, TOK], BF16, tag="stage",
                                      name=_nm("stg"))
                        nc.vector.tensor_mul(stg[:], pavs[0:64, :], rbc[:])
                        nc.sync.dma_start(attn_t[mj][64:128, :], stg[:])

                def av_unit(h, c):
                    mj = h // 2
                    ex = exps_tiles[mj]
                    if c == 0:
                        av_state[h] = ppv.tile([128, 512], F32, tag="pav",
                                               name=_nm("pav"))
                    pav = av_state[h]
                    vo = 65 * h
                    hs = h % 2
                    for p in (2 * c, 2 * c + 1):
                        nc.tensor.matmul(
                            pav[0:65, :], vaug[:, 2 * p:2 * p + 2, vo:vo + 65],
                            ex[:, 2 * p:2 * p + 2, hs, :],
                            start=(p == 0), stop=(p == 3),
                            perf_mode=DR)
                    if c == 1:
                        if hs == 1:
                            exps_tiles.pop(mj)
                        av_state.pop(h)
                        # free the PSUM bank fast; recip/bcast/mul lag off the
                        # SBUF copy without pacing the pav rotation
                        pavs = p2.tile([65, TOK], F32, tag="pavs", name=_nm("pavs"))
                        nc.vector.tensor_copy(pavs[:], pav[0:65, :])
                        nc.sync.dma_start(recips_d[h:h + 1, :], pavs[64:65, :])
                        # approx reciprocal is only valid on full-partition
                        # tiles: round-trip the denom row through DRAM as
                        # [128, 4] (130ns) instead of a 3.3us 1-lane recip
                        dT = bp.tile([128, 4], F32, tag="dT", name=_nm("dT"))
                        nc.gpsimd.dma_start(
                            dT[:], recips_d[h:h + 1, :].rearrange(
                                "o (p f) -> (o p) f", p=128))
                        nc.vector.reciprocal_approx_fast(dT[:], dT[:])
                        nc.gpsimd.dma_start(
                            recips_r[h:h + 1, :].rearrange(
                                "o (p f) -> (o p) f", p=128), dT[:])
                        rbc = p2.tile([64, TOK], F32, tag="lns", name=_nm("rbc"))
                        nc.sync.dma_start(
                            rbc[:],
                            recips_r[h:h + 1, None, :].to_broadcast((1, 64, TOK)))
                        av_pend[h] = (mj, pavs, rbc)
                        if (h - 1) in av_pend:
                            av_flush(h - 1)

                qpt = {}

                def q_unit(mg, c):
                    if c == 0:
                        qpt[mg] = pj()
                    wqh = wq_tiles[mg // 4]
                    m = mg % 4
                    for kk in (2 * c, 2 * c + 1):
                        nc.tensor.matmul(
                            qpt[mg][:], wqh[:, kk, m * 128:(m + 1) * 128],
                            rbn_in[:, kk, :],
                            start=(kk == 0), stop=(kk == KT - 1))
                    if c == 3:
                        nc.vector.tensor_scalar_add(
                            qt[:, mg, :], qpt.pop(mg)[:], bq_sb[:, mg:mg + 1])

                woh_t = [None, None]
                for c in range(4):
                    q_unit(0, c)
                for mg in range(KT):
                    if mg == 5:
                        woh_t[0] = wload("wo", wo[l, 0])
                    if mg == 7:
                        woh_t[1] = wload("wo", wo[l, 1])
                    units = []
                    if mg >= 1:
                        h0 = 2 * (mg - 1)
                        units += [("av", h0, 0), ("av", h0, 1),
                                  ("av", h0 + 1, 0), ("av", h0 + 1, 1)]
                    if mg + 1 < KT:
                        units += [("q", mg + 1, c) for c in range(4)]
                    s_units = [("s", mg, kt) for kt in range(KT)]
                    woven = []
                    oi = 0
                    for su in s_units:
                        woven.append(su)
                        if oi < len(units):
                            woven.append(units[oi]); oi += 1
                    woven += units[oi:]
                    for u in woven:
                        if u[0] == "s":
                            s_unit(u[1], u[2])
                        elif u[0] == "q":
                            q_unit(u[1], u[2])
                        else:
                            av_unit(u[1], u[2])
                for h in (14, 15):
                    for c in range(2):
                        av_unit(h, c)
                av_flush(15)
                if DEBUG and l == 0:
                    nc.sync.dma_start(dbg["dbg_qt"][:], qt[:])
                    for j in range(KT):
                        nc.sync.dma_start(dbg["dbg_attn"][:, j, :], attn_t[j][:])

                # --- O projection + residual; LN1 stats woven per m-tile ---
                # PSUM is seeded with bo_eff (+be2[l-1]); for l>=1 the drain
                # applies g2[l-1] to the x-hat residual stream.
                g1_sb = load_bias8(g1T, l)
                g2p_sb = load_bias8(g2T, l - 1) if l >= 1 else None
                # kk-outer over 8 accumulators (like fc2): every m-tile makes
                # progress on already-finished attn tiles, so the tail heads'
                # softmax chains overlap the bulk of the O matmuls
                o_a = psc.tile([128, 2, 512], F32, tag="sc", name=_nm("oa"))
                o_a2 = psc.tile([128, 2, 512], F32, tag="sc", name=_nm("oa2"))
                o_b = ppv.tile([128, 512], F32, tag="pav", name=_nm("ob"))
                o_c = ppv.tile([128, 512], F32, tag="pav", name=_nm("oc"))
                o_d = pj()
                o_e = pj()
                ops = [o_a[:, 0, :], o_a[:, 1, :], o_a2[:, 0, :], o_a2[:, 1, :],
                       o_b[:], o_c[:], o_d[:], o_e[:]]
                for mg in range(KT):
                    nc.tensor.matmul(
                        ops[mg], bor_sb[0:1, mg * 128:(mg + 1) * 128],
                        ones1t[0:1, :], start=True, stop=False)
                for kk in range(KT):
                    for mg in range(KT):
                        nc.tensor.matmul(
                            ops[mg],
                            woh_t[mg // 4][:, kk, (mg % 4) * 128:(mg % 4 + 1) * 128],
                            attn_t[kk][:],
                            start=False, stop=(kk == KT - 1))
                w1e_next = [wload("w1e", w1[l, 0])]
                for mg in range(KT):
                    if l == 0:
                        nc.vector.tensor_add(
                            x_cur[:, mg, :], ops[mg], x_cur[:, mg, :])
                    else:
                        nc.vector.scalar_tensor_tensor(
                            out=x_cur[:, mg, :], in0=x_cur[:, mg, :],
                            scalar=g2p_sb[:, mg:mg + 1], in1=ops[mg],
                            op0=OP.mult, op1=OP.add)
                ln1 = ln_begin()
                for mg in range(KT):
                    ln_stat(ln1, x_cur, mg)
                ln_final(ln1)
                if DEBUG and l == 0:
                    nc.sync.dma_start(dbg["dbg_r1"][:], x_cur[:])
                rb1 = ln1["rb"]
                # rbn1 = rb1 * rstd1 : fc1's folded weights consume this
                rbn1 = pxn.tile([128, KT, TOK], BF16, tag="xn", name=_nm("rbn1"))
                for kk in range(KT):
                    nc.vector.tensor_mul(rbn1[:, kk, :], rb1[:, kk, :],
                                         ln1["rs16"][:])

                # --- fc1 on the rstd-scaled residual (folded weights) ---
                b1_sb = bp.tile([128, FT], F32, tag="bias32", name=_nm("b1"))
                nc.sync.dma_start(b1_sb[:], b1T[l])
                b2r_sb = load_seed(b2r, l)
                g2_sb = load_bias8(g2T, l)
                ht = pbig.tile([128, FT, TOK], BF16, tag="big32", name=_nm("ht"))
                for e in range(8):
                    w1e = w1e_next[0]
                    if e < 7:
                        w1e_next[0] = wload("w1e", w1[l, e + 1])
                    for m in range(4):
                        fm = e * 4 + m
                        # 4-deep psum rotation (pj + ppv) keeps the PE fed
                        if fm % 2 == 0:
                            pt = pj()
                        else:
                            pt = ppv.tile([128, 512], F32, tag="pav",
                                          name=_nm("fpv"))
                        for kk in range(KT):
                            nc.tensor.matmul(
                                pt[:], w1e[:, kk, m * 128:(m + 1) * 128],
                                rbn1[:, kk, :],
                                start=(kk == 0), stop=(kk == KT - 1))
                        nc.scalar.activation(ht[:, fm, :], pt[:], AF.Relu,
                                             bias=b1_sb[:, fm:fm + 1])
                if DEBUG and l == 0:
                    nc.sync.dma_start(dbg["dbg_ht"][:], ht[:])
                # normalize the residual in place (r1 -> xhat1; g1/be1 are
                # folded into the fc2 drain/seed)
                for kk in range(KT):
                    ln_xhat_kk(ln1, x_cur, kk)

                # --- fc2 kk-outer over 8 accumulators; contiguous w2 loads ---
                f_a = psc.tile([128, 2, 512], F32, tag="sc", name=_nm("fa"))
                f_a2 = psc.tile([128, 2, 512], F32, tag="sc", name=_nm("fa2"))
                f_b = ppv.tile([128, 512], F32, tag="pav", name=_nm("fb"))
                f_c = ppv.tile([128, 512], F32, tag="pav", name=_nm("fc"))
                f_d = pj()
                f_e = pj()
                fps = [f_a[:, 0, :], f_a[:, 1, :], f_a2[:, 0, :], f_a2[:, 1, :],
                       f_b[:], f_c[:], f_d[:], f_e[:]]
                for mg in range(KT):
                    nc.tensor.matmul(
                        fps[mg], b2r_sb[0:1, mg * 128:(mg + 1) * 128],
                        ones1t[0:1, :], start=True, stop=False)

                # w2 row-blocks: split each load across both HWDGE queues and
                # run the prefetch two chunks deep so the PE never starves
                def w2load(kk):
                    t = w2p.tile([128, 1024], BF16, tag="w2c", name=_nm("w2c"))
                    nc.sync.dma_start(
                        t[:, 0:512], w2[l, kk * 128:(kk + 1) * 128, 0:512])
                    nc.scalar.dma_start(
                        t[:, 512:1024], w2[l, kk * 128:(kk + 1) * 128, 512:1024])
                    return t

                w2q = [w2load(0), w2load(1)]
                for kk in range(FT):
                    w2c = w2q.pop(0)
                    if kk < FT - 2:
                        w2q.append(w2load(kk + 2))
                    for mg in range(KT):
                        nc.tensor.matmul(
                            fps[mg], w2c[:, mg * 128:(mg + 1) * 128],
                            ht[:, kk, :],
                            start=False, stop=(kk == FT - 1))
                # all 8 accumulators must drain BEFORE ln_begin: the ln2 stats
                # tile takes f_a's PSUM slot (psc tag rotation)
                for mg in range(KT):
                    # r2 = xhat1*g1 + (h@w2 + b2 + be1)   (seeded PSUM)
                    nc.vector.scalar_tensor_tensor(
                        out=x_cur[:, mg, :], in0=x_cur[:, mg, :],
                        scalar=g1_sb[:, mg:mg + 1], in1=fps[mg],
                        op0=OP.mult, op1=OP.add)
                if DEBUG and l == 0:
                    nc.sync.dma_start(dbg["dbg_r2"][:], x_cur[:])
                ln2 = ln_begin()
                for mg in range(KT):
                    ln_stat(ln2, x_cur, mg)
                ln_final(ln2)
                rb2 = ln2["rb"]
                if l < L - 1:
                    rbn_cur = pxn.tile([128, KT, TOK], BF16, tag="xn",
                                       name=_nm("rbn2"))
                    for kk in range(KT):
                        nc.vector.tensor_mul(rbn_cur[:, kk, :], rb2[:, kk, :],
                                             ln2["rs16"][:])
                    for kk in range(KT):
                        ln_xhat_kk(ln2, x_cur, kk)
                    if DEBUG and l == 0:
                        nc.sync.dma_start(dbg["dbg_xh2"][:], x_cur[:])
                else:
                    be2_sb = load_bias8(be2T, l)
                    for kk in range(KT):
                        ln_xhat_kk(ln2, x_cur, kk)
                        nc.scalar.activation(
                            x_cur[:, kk, :], x_cur[:, kk, :], AF.Identity,
                            bias=be2_sb[:, kk:kk + 1], scale=g2_sb[:, kk:kk + 1])

            nc.sync.dma_start(
                xout.rearrange("(t p) n -> p t n", p=128), x_cur[:])

    return nc


MAXW = 1


def split_wait_overflow(nc, maxw=MAXW):
    """walrus in this toolchain rejects instructions with more than one sem
    wait; split excess waits onto preceding NoOp carriers on the same engine."""
    for f in nc.m.functions:
        for bb in f.blocks:
            if not any(i.sync_info and len(i.sync_info.on_wait) > maxw
                       for i in bb.instructions):
                continue
            newlist = []
            for inst in bb.instructions:
                si = inst.sync_info
                if si and len(si.on_wait) > maxw:
                    waits = list(si.on_wait)
                    extra, keep = waits[:-maxw], waits[-maxw:]
                    for i in range(0, len(extra), maxw):
                        newlist.append(mybir.InstNoOp(
                            name=f"{inst.name}-ws{i}", opcode="NoOp",
                            engine=inst.engine, debug=inst.debug, ins=[], outs=[],
                            sync_info=mybir.SyncInfo(
                                on_wait=extra[i:i + maxw], on_update=[]),
                        ))
                    inst.sync_info = mybir.SyncInfo(
                        on_wait=keep, on_update=list(si.on_update))
                newlist.append(inst)
            bb.instructions = newlist


def _get_nc():
    global _NC
    if _NC is None:
        _NC = _build_nc()
        # populate .instr bytes for extended InstISA (custom DVE ops);
        # raw Bass skips this codegen pass
        mybir.codegen_inst_isa_subclasses(_NC)
        split_wait_overflow(_NC)
    return _NC


def _to_bf16(a):
    return np.asarray(a, dtype=np.float32).astype(ml_dtypes.bfloat16)


def _bias_t(v, kt=KT):
    # [L, d] -> [L, 128, d//128] with column t = v[:, 128t:128t+128]
    v = np.asarray(v, dtype=np.float32)
    return np.ascontiguousarray(v.reshape(L, kt, 128).transpose(0, 2, 1))


def _fold(w, g, be):
    """LayerNorm fold: returns (w'', bias_delta) with
    x_hat @ w = (rstd*r) @ w'' + bias_delta  (per token rstd)."""
    wp = g[:, None] * w
    c = wp.sum(axis=0)
    return wp - c[None, :] / D, be @ w


def _tile_w(w, nchunk):
    """[L, D, n*512] -> [L, n, 128, KT, 512] matching the kernel's SBUF
    weight-tile layout so the DMA reads are contiguous."""
    Lw, Din, Dout = w.shape
    out = w.reshape(Lw, KT, 128, nchunk, 512).transpose(0, 3, 2, 1, 4)
    return np.ascontiguousarray(out)


def kernel(**inputs):
    nc = _get_nc()

    src = np.asarray(inputs["src"]).astype(np.int32).reshape(-1)      # [4096]
    src_mask = np.asarray(inputs["src_mask"]).astype(np.float32)      # [B,1,1,S]
    emb = np.asarray(inputs["emb"], dtype=np.float32)
    pe = np.asarray(inputs["pe"], dtype=np.float32)

    wq_f = np.asarray(inputs["wq"], dtype=np.float32).copy()
    wk_f = np.asarray(inputs["wk"], dtype=np.float32).copy()
    wv_f = np.asarray(inputs["wv"], dtype=np.float32).copy()
    wo_f = np.asarray(inputs["wo"], dtype=np.float32)
    w1_f = np.asarray(inputs["w1"], dtype=np.float32).copy()
    w2_f = np.asarray(inputs["w2"], dtype=np.float32)
    bq_f = np.asarray(inputs["bq"], dtype=np.float32).copy()
    bk_f = np.asarray(inputs["bk"], dtype=np.float32).copy()
    bv_f = np.asarray(inputs["bv"], dtype=np.float32).copy()
    bo_f = np.asarray(inputs["bo"], dtype=np.float32)
    b1_f = np.asarray(inputs["b1"], dtype=np.float32).copy()
    b2_f = np.asarray(inputs["b2"], dtype=np.float32)
    g1_f = np.asarray(inputs["g1"], dtype=np.float32)
    be1_f = np.asarray(inputs["be1"], dtype=np.float32)
    g2_f = np.asarray(inputs["g2"], dtype=np.float32)
    be2_f = np.asarray(inputs["be2"], dtype=np.float32)

    # fold LN1 into fc1 (all layers); fold LN2[l-1] into QKV[l] (l >= 1)
    for l in range(L):
        w1_f[l], d1 = _fold(w1_f[l], g1_f[l], be1_f[l])
        b1_f[l] = b1_f[l] + d1
        if l >= 1:
            g, be = g2_f[l - 1], be2_f[l - 1]
            wq_f[l], dq = _fold(wq_f[l], g, be)
            bq_f[l] = bq_f[l] + dq
            wk_f[l], dk_ = _fold(wk_f[l], g, be)
            bk_f[l] = bk_f[l] + dk_
            wv_f[l], dv = _fold(wv_f[l], g, be)
            bv_f[l] = bv_f[l] + dv
    # fold the V bias through the O projection: attn rows sum to 1, so
    # out = attn@(V + bv) @ wo + bo = attn@V@wo + (bv@wo + bo)
    bo_eff = np.stack([bo_f[l] + bv_f[l] @ wo_f[l] for l in range(L)])
    # O-proj PSUM seed: bo_eff plus the previous layer's LN2 shift (be2)
    bor_np = bo_eff.copy()
    for l in range(1, L):
        bor_np[l] = bor_np[l] + be2_f[l - 1]
    # fc2 PSUM seed: b2 plus this layer's LN1 shift (be1)
    b2r_np = b2_f + be1_f

    shared = {
        "emb": emb,
        "wq": _tile_w(_to_bf16(wq_f), 2), "wk": _tile_w(_to_bf16(wk_f), 2),
        "wv": _tile_w(_to_bf16(wv_f), 2), "wo": _tile_w(_to_bf16(wo_f), 2),
        "w1": _tile_w(_to_bf16(w1_f), 8), "w2": _to_bf16(w2_f),
        "bqT": _bias_t(bq_f), "bkT": _bias_t(bk_f),
        "b1T": _bias_t(b1_f, FT),
        "g1T": _bias_t(g1_f),
        "g2T": _bias_t(g2_f), "be2T": _bias_t(be2_f),
        "b2r": np.ascontiguousarray(_to_bf16(b2r_np).reshape(L, 1, D)),
        "bor": np.ascontiguousarray(_to_bf16(bor_np).reshape(L, 1, D)),
    }

    in_maps = []
    for c in range(NCORES):
        b = c // 2
        half = c % 2
        m = dict(shared)
        m["src"] = np.ascontiguousarray(
            src[c * TOK:(c + 1) * TOK].reshape(TOK, 1))
        m["peT"] = np.ascontiguousarray(
            pe[half * TOK:half * TOK + TOK, :D].T.astype(np.float32))
        mb = (src_mask[b, 0, 0, :] - 1.0) * 1e9
        own = slice(half * TOK, half * TOK + TOK)
        pair = slice((1 - half) * TOK, (1 - half) * TOK + TOK)
        mb_perm = np.concatenate([mb[own], mb[pair]])
        m["maskb"] = np.ascontiguousarray(
            mb_perm.reshape(KT, 128).T.astype(np.float32))
        o = 1 - half  # pair-local rank of the partner
        m["koidx"] = np.ascontiguousarray(
            (np.arange(512, dtype=np.int32) + o * 512).reshape(512, 1))
        # split V-AG layout: vag_out[hv] holds [rank0 rows, rank1 rows] of
        # 256-token slabs; partner token (mt*128+p) sits at o*256+(mt%2)*128+p
        vo = np.empty(TOK, dtype=np.int32)
        ar = np.arange(128, dtype=np.int32)
        for mt in range(4):
            vo[mt * 128:(mt + 1) * 128] = o * 256 + (mt % 2) * 128 + ar
        m["voidx"] = np.ascontiguousarray(vo.reshape(TOK, 1))
        in_maps.append(m)

    res = run_bass_kernel_spmd(nc, in_maps, list(range(NCORES)))
    out = np.empty((B * S, D), dtype=np.float32)
    for c in range(NCORES):
        out[c * TOK:(c + 1) * TOK] = res.results[c]["xout"].T
    return out.reshape(B, S, D)


# revision 47
# speedup vs baseline: 1.0279x; 1.0279x over previous
"""Trainium2 Bass kernel for a 6-layer transformer encoder (B=4, S=1024,
d_model=1024, 16 heads, d_ff=4096).

Sharding: token-parallel across 8 cores (B*S = 4096 tokens -> 512/core; each
core owns half of one batch element's sequence).  Per layer, pair-wise
AllGathers of K^T (bf16, 2 chunks) and V (fp8, vaug layout) let each core
rebuild K/V for its full batch element.

v5: the LayerNorm fold now pre-scales the residual once per layer
(rbn = rb * rstd, 8 cheap DVE bf16 muls) instead of applying the per-token
rstd at every PSUM drain.  All projection drains become single Scalar-engine
activations (Identity/Relu/Copy with per-partition bias), eliminating the
serialized 7.5us GpSimd tensor_scalar chain that dominated v4's critical
path.  The LN affine (g, be) is folded into the next projection: be rides a
K=1 ones-outer-product PSUM seed, g rides the drain's scalar_tensor_tensor.
Reciprocals use the ~5x faster approx-NR custom DVE op.  LN2's x-hat is kept
unscaled in the residual stream; the final layer applies g2/be2 on the way
out.
"""

import sys
import os

for _p in ("/opt/trn_rl_repo", "/root/.axon_site/_ro/trn_rl_repo"):
    if os.path.isdir(_p) and _p not in sys.path:
        sys.path.insert(0, _p)

import numpy as np
import ml_dtypes

import concourse.bass as bass
import concourse.mybir as mybir
import concourse.tile as tile
from concourse.bass_utils import run_bass_kernel_spmd
from concourse.masks import make_identity

VOCAB, D, H, DFF, L = 32000, 1024, 16, 4096, 6
B, S = 4, 1024
DK = D // H              # 64
NCORES = 8
TOK = (B * S) // NCORES  # 512 tokens per core
KT = D // 128            # 8
FT = DFF // 128          # 32
EPS = 1e-5
VS = 16.0                # fp8 V pre-scale (ones col carries the same scale)

F32 = mybir.dt.float32
BF16 = mybir.dt.bfloat16
FP8 = mybir.dt.float8e4
I32 = mybir.dt.int32
AF = mybir.ActivationFunctionType
OP = mybir.AluOpType
DR = mybir.MatmulPerfMode.DoubleRow

_NC = None
DEBUG = False

PAIRS = [[2 * i, 2 * i + 1] for i in range(NCORES // 2)]


def _build_nc():
    nc = bass.Bass("TRN2", target_bir_lowering=False, debug=False, num_devices=NCORES)

    emb = nc.dram_tensor("emb", [VOCAB, D], F32, kind="ExternalInput")
    src = nc.dram_tensor("src", [TOK, 1], I32, kind="ExternalInput")
    peT = nc.dram_tensor("peT", [D, TOK], F32, kind="ExternalInput")
    maskb = nc.dram_tensor("maskb", [128, KT], F32, kind="ExternalInput")
    koidx = nc.dram_tensor("koidx", [512, 1], I32, kind="ExternalInput")
    voidx = nc.dram_tensor("voidx", [TOK, 1], I32, kind="ExternalInput")
    # projection weights pre-tiled host-side to [L, chunk, 128, KT, cols]
    # so every weight DMA is a fully contiguous read
    wq = nc.dram_tensor("wq", [L, 2, 128, KT, 512], BF16, kind="ExternalInput")
    wk = nc.dram_tensor("wk", [L, 2, 128, KT, 512], BF16, kind="ExternalInput")
    wv = nc.dram_tensor("wv", [L, 2, 128, KT, 512], BF16, kind="ExternalInput")
    wo = nc.dram_tensor("wo", [L, 2, 128, KT, 512], BF16, kind="ExternalInput")
    w1 = nc.dram_tensor("w1", [L, 8, 128, KT, 512], BF16, kind="ExternalInput")
    w2 = nc.dram_tensor("w2", [L, DFF, D], BF16, kind="ExternalInput")
    bqT = nc.dram_tensor("bqT", [L, 128, KT], F32, kind="ExternalInput")
    bkT = nc.dram_tensor("bkT", [L, 128, KT], F32, kind="ExternalInput")
    b1T = nc.dram_tensor("b1T", [L, 128, FT], F32, kind="ExternalInput")
    g1T = nc.dram_tensor("g1T", [L, 128, KT], F32, kind="ExternalInput")
    g2T = nc.dram_tensor("g2T", [L, 128, KT], F32, kind="ExternalInput")
    be2T = nc.dram_tensor("be2T", [L, 128, KT], F32, kind="ExternalInput")
    # K=1 PSUM-seed rows: fc2 gets b2+be1, O-proj gets bo_eff(+be2[l-1])
    b2r = nc.dram_tensor("b2r", [L, 1, D], BF16, kind="ExternalInput")
    bor = nc.dram_tensor("bor", [L, 1, D], BF16, kind="ExternalInput")
    xout = nc.dram_tensor("xout", [D, TOK], F32, kind="ExternalOutput")
    dbg = {}
    if DEBUG:
        for nm, shp, dt in [
            ("dbg_x0", [128, KT, TOK], F32),     # post-embedding x
            ("dbg_ktl", [128, KT, TOK], BF16),   # K proj (own half)
            ("dbg_qt", [128, KT, TOK], BF16),    # Q proj
            ("dbg_attn", [128, KT, TOK], BF16),  # softmax @ V
            ("dbg_r1", [128, KT, TOK], F32),     # post O-drain residual
            ("dbg_ht", [128, FT, TOK], BF16),    # fc1 relu out
            ("dbg_r2", [128, KT, TOK], F32),     # post fc2 residual
            ("dbg_xh2", [128, KT, TOK], F32),    # xhat2 end of layer 0
        ]:
            dbg[nm] = nc.dram_tensor(nm, shp, dt, kind="ExternalOutput")

    from contextlib import ExitStack
    with tile.TileContext(nc) as tc:
        with ExitStack() as _es:
            def _pool(**kw):
                return _es.enter_context(tc.tile_pool(**kw))
            cpool = _pool(name="cpool", bufs=1)
            wp = _pool(name="wp", bufs=2)        # QKVO/fc1 weight chunks
            w2p = _pool(name="w2p", bufs=3)      # fc2 weight row-blocks
            pbig = _pool(name="pbig", bufs=1)    # ht
            ppe = _pool(name="ppe", bufs=1)      # peT (embedding only)
            px = _pool(name="px", bufs=1)        # f32 residual (in-place)
            pxb = _pool(name="pxb", bufs=2)      # bf16 pre-norm rb ping-pong
            pxn = _pool(name="pxn", bufs=2)      # bf16 rstd-scaled rbn
            pat = _pool(name="pat", bufs=1)      # ktl/kto/qt/attn
            pexp = _pool(name="pexp", bufs=2)    # exps per head-pair
            p2 = _pool(name="p2", bufs=2)        # transients (sq/stages)
            prs = _pool(name="prs", bufs=2)      # rstd rows
            pv3 = _pool(name="pv3", bufs=3)      # pavs (3-deep: 2-stage defer)
            bp = _pool(name="bp", bufs=8)        # bias rows
            psd = _pool(name="psd", bufs=1)      # K=1 PSUM-seed rows
            psc = _pool(name="psc", bufs=2, space="PSUM")   # 2-bank tiles
            ppv = _pool(name="ppv", bufs=2, space="PSUM")   # 1-bank tiles
            ppj = _pool(name="ppj", bufs=2, space="PSUM")   # proj psums
            dram = _pool(name="dram", bufs=2, space="DRAM")
            _uid = [0]

            def _nm(tag):
                _uid[0] += 1
                return f"{tag}_{_uid[0]}"

            ident = cpool.tile([128, 128], BF16, tag="ident", name=_nm("ident"))
            make_identity(nc, ident[:])
            onesk = cpool.tile([128, 128], BF16, tag="onesk", name=_nm("onesk"))
            nc.vector.memset(onesk[:], 1.0 / D)
            ones1t = cpool.tile([1, TOK], BF16, tag="ones1t", name=_nm("ones1t"))
            nc.vector.memset(ones1t[:], 1.0)
            maskb_sb = cpool.tile([128, KT], F32, tag="maskb", name=_nm("maskb"))
            nc.sync.dma_start(maskb_sb[:], maskb[:])
            eps_sb = cpool.tile([128, 1], F32, tag="eps", name=_nm("eps"))
            nc.vector.memset(eps_sb[:], EPS)
            # vaug: [tok-part, kt, head*65] fp8; col 64 of each head block is
            # the constant VS used to accumulate the softmax denominator.
            vaug = cpool.tile([128, KT, H * 65], FP8, tag="vaug", name=_nm("vaug"))
            nc.vector.memset(
                vaug[:].rearrange("p t (h x) -> p t h x", x=65)[:, :, :, 64:65], VS)

            def pj():
                return ppj.tile([128, 512], F32, tag="pj", name=_nm("pj"))

            def load_bias8(t, l):
                b = bp.tile([128, KT], F32, tag="bias8", name=_nm("bias8"))
                nc.sync.dma_start(b[:], t[l])
                return b

            def load_seed(t, l):
                s = psd.tile([1, D], BF16, tag="seedrow", name=_nm("seed"))
                nc.sync.dma_start(s[:], t[l])
                return s

            # ---------------- embedding ----------------
            peT_sb = ppe.tile([128, KT, TOK], F32, tag="peT", name=_nm("peT"))
            nc.sync.dma_start(peT_sb[:], peT.rearrange("(t p) n -> p t n", p=128))
            x_cur = px.tile([128, KT, TOK], F32, tag="x", name=_nm("x"))
            for blk in range(TOK // 128):
                idx_t = p2.tile([128, 1], I32, tag="idx", name=_nm("idx"))
                nc.sync.dma_start(idx_t[:], src[blk * 128:(blk + 1) * 128, :])
                gat = p2.tile([128, D], F32, tag="bfs", name=_nm("gat"))
                nc.gpsimd.indirect_dma_start(
                    out=gat[:], out_offset=None, in_=emb[:],
                    in_offset=bass.IndirectOffsetOnAxis(ap=idx_t[:, :1], axis=0),
                )
                gatb = p2.tile([128, D], BF16, tag="lns", name=_nm("gatb"))
                nc.scalar.activation(gatb[:], gat[:], AF.Copy)
                for kt in range(KT):
                    tp = ppj.tile([128, 512], BF16, tag="pj", name=_nm("tp"))
                    nc.tensor.transpose(tp[:, :128], gatb[:, kt * 128:(kt + 1) * 128],
                                        ident[:])
                    nc.vector.scalar_tensor_tensor(
                        out=x_cur[:, kt, blk * 128:(blk + 1) * 128],
                        in0=tp[:, :128], scalar=32.0,
                        in1=peT_sb[:, kt, blk * 128:(blk + 1) * 128],
                        op0=OP.mult, op1=OP.add,
                    )
            rbn_cur = pxn.tile([128, KT, TOK], BF16, tag="xn", name=_nm("xn0"))
            for kk in range(KT):
                nc.scalar.activation(rbn_cur[:, kk, :], x_cur[:, kk, :], AF.Copy)
            if DEBUG:
                nc.sync.dma_start(dbg["dbg_x0"][:], x_cur[:])

            # ------------- layer norm pieces -------------
            def ln_begin():
                st = psc.tile([128, 2, 512], F32, tag="sc", name=_nm("lnst"))
                rb = pxb.tile([128, KT, TOK], BF16, tag="xb", name=_nm("rb"))
                return {"st": st, "rb": rb}

            def ln_stat(s, r, mg):
                nc.scalar.activation(s["rb"][:, mg, :], r[:, mg, :], AF.Copy)
                sq = p2.tile([128, TOK], BF16, tag="sq1", name=_nm("sq"))
                nc.vector.tensor_mul(sq[:], s["rb"][:, mg, :],
                                     s["rb"][:, mg, :])
                nc.tensor.matmul(s["st"][:, 0, :], onesk[:], s["rb"][:, mg, :],
                                 start=(mg == 0), stop=(mg == KT - 1))
                nc.tensor.matmul(s["st"][:, 1, :], onesk[:], sq[:],
                                 start=(mg == 0), stop=(mg == KT - 1))

            def ln_final(s):
                # rstd = sqrt(1/(var+eps)): the approx-reciprocal runs on the
                # raw variance so both the f32 and bf16 sqrt taps come straight
                # off one DVE chain (shorter than sqrt->recip->cast)
                msq = p2.tile([128, TOK], F32, tag="lns", name=_nm("msq"))
                nc.scalar.activation(msq[:], s["st"][:, 0, :], AF.Square)
                inv = p2.tile([128, TOK], F32, tag="lns", name=_nm("inv"))
                nc.vector.scalar_tensor_tensor(
                    out=inv[:], in0=s["st"][:, 1, :], scalar=eps_sb[:, 0:1],
                    in1=msq[:], op0=OP.add, op1=OP.subtract)
                nc.vector.reciprocal_approx_fast(inv[:], inv[:])
                rstd = prs.tile([128, TOK], F32, tag="rstd", name=_nm("rstd"))
                nc.scalar.activation(rstd[:], inv[:], AF.Sqrt)
                rs16 = prs.tile([128, TOK], BF16, tag="rs16", name=_nm("rs16"))
                nc.scalar.activation(rs16[:], inv[:], AF.Sqrt)
                s["rstd"] = rstd
                s["rs16"] = rs16

            def ln_xhat_kk(s, r, kk):
                # in-place normalize of the residual stream (r -> x-hat,
                # WITHOUT the g/be affine - that is folded downstream)
                nc.vector.tensor_sub(r[:, kk, :], r[:, kk, :], s["st"][:, 0, :])
                nc.vector.tensor_mul(r[:, kk, :], r[:, kk, :], s["rstd"][:])

            # ---------------- layers ----------------
            for l in range(L):
                bk_sb = load_bias8(bkT, l)
                bq_sb = load_bias8(bqT, l)
                bor_sb = load_seed(bor, l)
                rbn_in = rbn_cur        # rstd-scaled residual (or embedding)

                # --- K projection + chunked pair-AllGather of K^T ---
                # weight loads ride the ACT queue (nc.scalar) one phase ahead
                # of their consumers so the PE never waits on LDWEIGHTS input
                def wload(t, src):
                    w = wp.tile([128, KT, 512], BF16, tag="wproj", name=_nm(t))
                    nc.scalar.dma_start(w[:], src)
                    return w

                wkh_t = [wload("wk", wk[l, h]) for h in range(2)]
                wvh_t = [None, None]
                wq_tiles = {}
                ktl = pat.tile([128, KT, TOK], BF16, tag="ktl", name=_nm("ktl"))
                kag_in = dram.tile([D, TOK], BF16, tag="kag_in", name=_nm("kag_in"))
                kag_out = [
                    dram.tile([2 * 512, TOK], BF16, tag=f"kag_out{c}",
                              name=_nm("kag_out")) for c in range(2)]
                kto = pat.tile([128, KT, TOK], BF16, tag="kto", name=_nm("kto"))
                for half in range(2):
                    wkh = wkh_t[half]
                    for m in range(4):
                        mg = half * 4 + m
                        pt = pj()
                        for kk in range(KT):
                            nc.tensor.matmul(
                                pt[:], wkh[:, kk, m * 128:(m + 1) * 128],
                                rbn_in[:, kk, :],
                                start=(kk == 0), stop=(kk == KT - 1))
                        nc.scalar.activation(ktl[:, mg, :], pt[:], AF.Identity,
                                             bias=bk_sb[:, mg:mg + 1])
                        nc.sync.dma_start(
                            kag_in[mg * 128:(mg + 1) * 128, :], ktl[:, mg, :])
                    nc.gpsimd.collective_compute(
                        "AllGather", OP.bypass,
                        ins=[kag_in[half * 512:(half + 1) * 512, :]],
                        outs=[kag_out[half][:]],
                        replica_groups=PAIRS,
                    )
                    wvh_t[half] = wload("wv", wv[l, half])
                    # partner-K gathers ride right behind this half's AG
                    for g in range(4 * half, 4 * half + 4):
                        kidx = bp.tile([128, 1], I32, tag="koidx",
                                       name=_nm("koidx"))
                        nc.sync.dma_start(
                            kidx[:], koidx[(g % 4) * 128:(g % 4) * 128 + 128, :])
                        nc.gpsimd.indirect_dma_start(
                            out=kto[:, g, :], out_offset=None,
                            in_=kag_out[g // 4][:],
                            in_offset=bass.IndirectOffsetOnAxis(
                                ap=kidx[:, :1], axis=0),
                        )
                if DEBUG and l == 0:
                    nc.sync.dma_start(dbg["dbg_ktl"][:], ktl[:])

                # --- V projection into vaug (own slots) + fp8 AllGather ---
                # mt-outer so each 128-token row block is complete (both head
                # halves) early; the AG is split in two so partner-V gathers
                # start at the V-phase midpoint instead of after it
                vag_in = dram.tile([TOK, H * 65], FP8, tag="vag_in", name=_nm("vag_in"))
                vag_out = [
                    dram.tile([2 * 256, H * 65], FP8, tag=f"vag_out{c}",
                              name=_nm("vag_out")) for c in range(2)]
                for mt in range(4):
                    for half in range(2):
                        wvh = wvh_t[half]
                        pt = pj()
                        for kk in range(KT):
                            nc.tensor.matmul(
                                pt[:], rbn_in[:, kk, mt * 128:(mt + 1) * 128],
                                wvh[:, kk, :],
                                start=(kk == 0), stop=(kk == KT - 1))
                        vdst = vaug[:, mt, :].rearrange(
                            "p (h x) -> p h x", x=65)[:, half * 8:(half + 1) * 8, 0:64]
                        vsrc = pt[:].rearrange("p (h c) -> p h c", c=64)
                        nc.scalar.activation(vdst, vsrc, AF.Copy, scale=VS)
                    nc.sync.dma_start(
                        vag_in[mt * 128:(mt + 1) * 128, :], vaug[:, mt, :])
                    if mt % 2 == 1:
                        hv = mt // 2
                        nc.gpsimd.collective_compute(
                            "AllGather", OP.bypass,
                            ins=[vag_in[hv * 256:(hv + 1) * 256, :]],
                            outs=[vag_out[hv][:]],
                            replica_groups=PAIRS,
                        )
                        if hv == 0:
                            wq_tiles[0] = wload("wq", wq[l, 0])
                        for mt2 in (2 * hv, 2 * hv + 1):
                            vidx = bp.tile([128, 1], I32, tag="voidx",
                                           name=_nm("voidx"))
                            nc.sync.dma_start(
                                vidx[:], voidx[mt2 * 128:(mt2 + 1) * 128, :])
                            nc.gpsimd.indirect_dma_start(
                                out=vaug[:, 4 + mt2, :], out_offset=None,
                                in_=vag_out[hv][:],
                                in_offset=bass.IndirectOffsetOnAxis(
                                    ap=vidx[:, :1], axis=0),
                            )
                wq_tiles[1] = wload("wq", wq[l, 1])

                # --- Q projection woven with attention at 2-MM granularity ---
                qt = pat.tile([128, KT, TOK], BF16, tag="qt", name=_nm("qt"))
                # per-m-tile attn tiles: O-proj's kk-th matmul depends only on
                # tile kk, so the tail heads' softmax chains overlap O-proj
                attn_t = [pat.tile([128, TOK], BF16, tag=f"attn{j}",
                                   name=_nm("attn")) for j in range(KT)]
                recips_d = dram.tile([H, TOK], F32, tag="recips_d", name=_nm("recd"))
                recips_r = dram.tile([H, TOK], F32, tag="recips_r", name=_nm("recr"))
                exps_tiles = {}

                def kt_lhs(kt, mj, prow):
                    if kt < 4:
                        return ktl[prow:prow + 64, mj, kt * 128:(kt + 1) * 128]
                    return kto[prow:prow + 64, mj, (kt - 4) * 128:(kt - 3) * 128]

                def s_unit(mj, kt):
                    # one kt-block of scores for BOTH heads of m-tile mj; a
                    # single exp instruction covers the pair (same mask col)
                    if kt == 0:
                        exps_tiles[mj] = pexp.tile([128, KT, 2, TOK], FP8,
                                                   tag="exps", name=_nm("ex"))
                    ex = exps_tiles[mj]
                    st = psc.tile([128, 2, 512], F32, tag="sc", name=_nm("sc"))
                    for j in range(2):
                        prow = j * 64
                        nc.tensor.matmul(
                            st[:, j, :], kt_lhs(kt, mj, prow),
                            qt[prow:prow + 64, mj, :],
                            start=True, stop=True)
                    nc.scalar.activation(
                        ex[:, kt, :, :], st[:, :, :], AF.Exp,
                        scale=DK ** -0.5, bias=maskb_sb[:, kt:kt + 1])

                av_state = {}
                av_p1 = {}
                av_p2 = {}

                def av_stage1(h):
                    # one head-slot after the dT reload was issued: the recip
                    # never head-of-line-blocks the DVE queue on that DMA
                    mj, pavs, dT = av_p1.pop(h)
                    nc.vector.reciprocal_approx_fast(dT[:], dT[:])
                    nc.gpsimd.dma_start(
                        recips_r[h:h + 1, :].rearrange(
                            "o (p f) -> (o p) f", p=128), dT[:])
                    rbc = p2.tile([64, TOK], F32, tag="lns", name=_nm("rbc"))
                    nc.sync.dma_start(
                        rbc[:],
                        recips_r[h:h + 1, None, :].to_broadcast((1, 64, TOK)))
                    av_p2[h] = (mj, pavs, rbc)

                def av_stage2(h):
                    # and the attn mul one further slot later, once rbc landed
                    mj, pavs, rbc = av_p2.pop(h)
                    if h % 2 == 0:
                        nc.vector.tensor_mul(
                            attn_t[mj][0:64, :], pavs[0:64, :], rbc[:])
                    else:
                        stg = p2.tile([64, TOK], BF16, tag="stage",
                                      name=_nm("stg"))
                        nc.vector.tensor_mul(stg[:], pavs[0:64, :], rbc[:])
                        nc.sync.dma_start(attn_t[mj][64:128, :], stg[:])

                def av_unit(h, c):
                    mj = h // 2
                    ex = exps_tiles[mj]
                    if c == 0:
                        av_state[h] = ppv.tile([128, 512], F32, tag="pav",
                                               name=_nm("pav"))
                    pav = av_state[h]
                    vo = 65 * h
                    hs = h % 2
                    for p in (2 * c, 2 * c + 1):
                        nc.tensor.matmul(
                            pav[0:65, :], vaug[:, 2 * p:2 * p + 2, vo:vo + 65],
                            ex[:, 2 * p:2 * p + 2, hs, :],
                            start=(p == 0), stop=(p == 3),
                            perf_mode=DR)
                    if c == 1:
                        if hs == 1:
                            exps_tiles.pop(mj)
                        av_state.pop(h)
                        # free the PSUM bank fast; the rest of the softmax
                        # chain lags 1-2 head-slots behind the SBUF copy
                        pavs = pv3.tile([65, TOK], F32, tag="pavs", name=_nm("pavs"))
                        nc.vector.tensor_copy(pavs[:], pav[0:65, :])
                        nc.sync.dma_start(recips_d[h:h + 1, :], pavs[64:65, :])
                        # approx reciprocal is only valid on full-partition
                        # tiles: round-trip the denom row through DRAM as
                        # [128, 4] (130ns) instead of a 3.3us 1-lane recip
                        dT = bp.tile([128, 4], F32, tag="dT", name=_nm("dT"))
                        nc.gpsimd.dma_start(
                            dT[:], recips_d[h:h + 1, :].rearrange(
                                "o (p f) -> (o p) f", p=128))
                        av_p1[h] = (mj, pavs, dT)
                        if (h - 1) in av_p1:
                            av_stage1(h - 1)
                        if (h - 2) in av_p2:
                            av_stage2(h - 2)

                qpt = {}

                def q_unit(mg, c):
                    if c == 0:
                        qpt[mg] = pj()
                    wqh = wq_tiles[mg // 4]
                    m = mg % 4
                    for kk in (2 * c, 2 * c + 1):
                        nc.tensor.matmul(
                            qpt[mg][:], wqh[:, kk, m * 128:(m + 1) * 128],
                            rbn_in[:, kk, :],
                            start=(kk == 0), stop=(kk == KT - 1))
                    if c == 3:
                        nc.vector.tensor_scalar_add(
                            qt[:, mg, :], qpt.pop(mg)[:], bq_sb[:, mg:mg + 1])

                woh_t = [None, None]
                for c in range(4):
                    q_unit(0, c)
                for mg in range(KT):
                    if mg == 5:
                        woh_t[0] = wload("wo", wo[l, 0])
                    if mg == 7:
                        woh_t[1] = wload("wo", wo[l, 1])
                    units = []
                    if mg >= 1:
                        h0 = 2 * (mg - 1)
                        units += [("av", h0, 0), ("av", h0, 1),
                                  ("av", h0 + 1, 0), ("av", h0 + 1, 1)]
                    if mg + 1 < KT:
                        units += [("q", mg + 1, c) for c in range(4)]
                    s_units = [("s", mg, kt) for kt in range(KT)]
                    woven = []
                    oi = 0
                    for su in s_units:
                        woven.append(su)
                        if oi < len(units):
                            woven.append(units[oi]); oi += 1
                    woven += units[oi:]
                    for u in woven:
                        if u[0] == "s":
                            s_unit(u[1], u[2])
                        elif u[0] == "q":
                            q_unit(u[1], u[2])
                        else:
                            av_unit(u[1], u[2])
                for h in (14, 15):
                    for c in range(2):
                        av_unit(h, c)
                av_stage1(15)
                av_stage2(14)
                av_stage2(15)
                if DEBUG and l == 0:
                    nc.sync.dma_start(dbg["dbg_qt"][:], qt[:])
                    for j in range(KT):
                        nc.sync.dma_start(dbg["dbg_attn"][:, j, :], attn_t[j][:])

                # --- O projection + residual; LN1 stats woven per m-tile ---
                # PSUM is seeded with bo_eff (+be2[l-1]); for l>=1 the drain
                # applies g2[l-1] to the x-hat residual stream.
                g1_sb = load_bias8(g1T, l)
                g2p_sb = load_bias8(g2T, l - 1) if l >= 1 else None
                # kk-outer over 4 accumulators, two passes (mg 0-3 on psc with
                # woh0, mg 4-7 on ppv+ppj with woh1): the attention tail's
                # softmax chains overlap the O matmuls, and woh0 dies at the
                # pass boundary so the first fc1 weight chunk loads early
                o_a = psc.tile([128, 2, 512], F32, tag="sc", name=_nm("oa"))
                o_a2 = psc.tile([128, 2, 512], F32, tag="sc", name=_nm("oa2"))
                o_b = ppv.tile([128, 512], F32, tag="pav", name=_nm("ob"))
                o_c = ppv.tile([128, 512], F32, tag="pav", name=_nm("oc"))
                o_d = pj()
                o_e = pj()
                ops = [o_a[:, 0, :], o_a[:, 1, :], o_a2[:, 0, :], o_a2[:, 1, :],
                       o_b[:], o_c[:], o_d[:], o_e[:]]
                for half in range(2):
                    mgs = range(4 * half, 4 * half + 4)
                    for mg in mgs:
                        nc.tensor.matmul(
                            ops[mg], bor_sb[0:1, mg * 128:(mg + 1) * 128],
                            ones1t[0:1, :], start=True, stop=False)
                    for kk in range(KT):
                        for mg in mgs:
                            nc.tensor.matmul(
                                ops[mg],
                                woh_t[half][:, kk, (mg % 4) * 128:(mg % 4 + 1) * 128],
                                attn_t[kk][:],
                                start=False, stop=(kk == KT - 1))
                    if half == 0:
                        w1e_next = [wload("w1e", w1[l, 0])]

                def o_drain(mg):
                    if l == 0:
                        nc.vector.tensor_add(
                            x_cur[:, mg, :], ops[mg], x_cur[:, mg, :])
                    else:
                        nc.vector.scalar_tensor_tensor(
                            out=x_cur[:, mg, :], in0=x_cur[:, mg, :],
                            scalar=g2p_sb[:, mg:mg + 1], in1=ops[mg],
                            op0=OP.mult, op1=OP.add)

                # drains 0-1 must precede ln_begin (its stats tile takes o_a's
                # PSUM banks); the rest interleave with the stats
                o_drain(0)
                o_drain(1)
                ln1 = ln_begin()
                for mg in range(2, KT):
                    o_drain(mg)
                    ln_stat(ln1, x_cur, mg - 2)
                ln_stat(ln1, x_cur, 6)
                ln_stat(ln1, x_cur, 7)
                ln_final(ln1)
                if DEBUG and l == 0:
                    nc.sync.dma_start(dbg["dbg_r1"][:], x_cur[:])
                rb1 = ln1["rb"]
                # rbn1 = rb1 * rstd1 : fc1's folded weights consume this
                rbn1 = pxn.tile([128, KT, TOK], BF16, tag="xn", name=_nm("rbn1"))
                for kk in range(KT):
                    nc.vector.tensor_mul(rbn1[:, kk, :], rb1[:, kk, :],
                                         ln1["rs16"][:])

                # --- fc1 on the rstd-scaled residual (folded weights) ---
                b1_sb = bp.tile([128, FT], F32, tag="bias32", name=_nm("b1"))
                nc.sync.dma_start(b1_sb[:], b1T[l])
                b2r_sb = load_seed(b2r, l)
                g2_sb = load_bias8(g2T, l)
                ht = pbig.tile([128, FT, TOK], BF16, tag="big32", name=_nm("ht"))
                for e in range(8):
                    w1e = w1e_next[0]
                    if e < 7:
                        w1e_next[0] = wload("w1e", w1[l, e + 1])
                    for m in range(4):
                        fm = e * 4 + m
                        # 4-deep psum rotation (pj + ppv) keeps the PE fed
                        if fm % 2 == 0:
                            pt = pj()
                        else:
                            pt = ppv.tile([128, 512], F32, tag="pav",
                                          name=_nm("fpv"))
                        for kk in range(KT):
                            nc.tensor.matmul(
                                pt[:], w1e[:, kk, m * 128:(m + 1) * 128],
                                rbn1[:, kk, :],
                                start=(kk == 0), stop=(kk == KT - 1))
                        nc.scalar.activation(ht[:, fm, :], pt[:], AF.Relu,
                                             bias=b1_sb[:, fm:fm + 1])
                if DEBUG and l == 0:
                    nc.sync.dma_start(dbg["dbg_ht"][:], ht[:])
                # normalize the residual in place (r1 -> xhat1; g1/be1 are
                # folded into the fc2 drain/seed)
                for kk in range(KT):
                    ln_xhat_kk(ln1, x_cur, kk)

                # --- fc2 kk-outer over 8 accumulators; contiguous w2 loads ---
                f_a = psc.tile([128, 2, 512], F32, tag="sc", name=_nm("fa"))
                f_a2 = psc.tile([128, 2, 512], F32, tag="sc", name=_nm("fa2"))
                f_b = ppv.tile([128, 512], F32, tag="pav", name=_nm("fb"))
                f_c = ppv.tile([128, 512], F32, tag="pav", name=_nm("fc"))
                f_d = pj()
                f_e = pj()
                fps = [f_a[:, 0, :], f_a[:, 1, :], f_a2[:, 0, :], f_a2[:, 1, :],
                       f_b[:], f_c[:], f_d[:], f_e[:]]
                for mg in range(KT):
                    nc.tensor.matmul(
                        fps[mg], b2r_sb[0:1, mg * 128:(mg + 1) * 128],
                        ones1t[0:1, :], start=True, stop=False)

                # w2 row-blocks: split each load across both HWDGE queues and
                # run the prefetch two chunks deep so the PE never starves
                def w2load(kk):
                    t = w2p.tile([128, 1024], BF16, tag="w2c", name=_nm("w2c"))
                    nc.sync.dma_start(
                        t[:, 0:512], w2[l, kk * 128:(kk + 1) * 128, 0:512])
                    nc.scalar.dma_start(
                        t[:, 512:1024], w2[l, kk * 128:(kk + 1) * 128, 512:1024])
                    return t

                w2q = [w2load(0), w2load(1)]
                for kk in range(FT):
                    w2c = w2q.pop(0)
                    if kk < FT - 2:
                        w2q.append(w2load(kk + 2))
                    for mg in range(KT):
                        nc.tensor.matmul(
                            fps[mg], w2c[:, mg * 128:(mg + 1) * 128],
                            ht[:, kk, :],
                            start=False, stop=(kk == FT - 1))
                def f_drain(mg):
                    # r2 = xhat1*g1 + (h@w2 + b2 + be1)   (seeded PSUM)
                    nc.vector.scalar_tensor_tensor(
                        out=x_cur[:, mg, :], in0=x_cur[:, mg, :],
                        scalar=g1_sb[:, mg:mg + 1], in1=fps[mg],
                        op0=OP.mult, op1=OP.add)

                # drains 0-1 free f_a's banks before ln_begin claims them;
                # the remaining drains interleave with the stats
                f_drain(0)
                f_drain(1)
                ln2 = ln_begin()
                for mg in range(2, KT):
                    f_drain(mg)
                    ln_stat(ln2, x_cur, mg - 2)
                ln_stat(ln2, x_cur, 6)
                ln_stat(ln2, x_cur, 7)
                if DEBUG and l == 0:
                    nc.sync.dma_start(dbg["dbg_r2"][:], x_cur[:])
                ln_final(ln2)
                rb2 = ln2["rb"]
                if l < L - 1:
                    rbn_cur = pxn.tile([128, KT, TOK], BF16, tag="xn",
                                       name=_nm("rbn2"))
                    for kk in range(KT):
                        nc.vector.tensor_mul(rbn_cur[:, kk, :], rb2[:, kk, :],
                                             ln2["rs16"][:])
                    for kk in range(KT):
                        ln_xhat_kk(ln2, x_cur, kk)
                    if DEBUG and l == 0:
                        nc.sync.dma_start(dbg["dbg_xh2"][:], x_cur[:])
                else:
                    be2_sb = load_bias8(be2T, l)
                    for kk in range(KT):
                        ln_xhat_kk(ln2, x_cur, kk)
                        nc.scalar.activation(
                            x_cur[:, kk, :], x_cur[:, kk, :], AF.Identity,
                            bias=be2_sb[:, kk:kk + 1], scale=g2_sb[:, kk:kk + 1])

            nc.sync.dma_start(
                xout.rearrange("(t p) n -> p t n", p=128), x_cur[:])

    return nc


MAXW = 1


def split_wait_overflow(nc, maxw=MAXW):
    """walrus in this toolchain rejects instructions with more than one sem
    wait; split excess waits onto preceding NoOp carriers on the same engine."""
    for f in nc.m.functions:
        for bb in f.blocks:
            if not any(i.sync_info and len(i.sync_info.on_wait) > maxw
                       for i in bb.instructions):
                continue
            newlist = []
            for inst in bb.instructions:
                si = inst.sync_info
                if si and len(si.on_wait) > maxw:
                    waits = list(si.on_wait)
                    extra, keep = waits[:-maxw], waits[-maxw:]
                    for i in range(0, len(extra), maxw):
                        newlist.append(mybir.InstNoOp(
                            name=f"{inst.name}-ws{i}", opcode="NoOp",
                            engine=inst.engine, debug=inst.debug, ins=[], outs=[],
                            sync_info=mybir.SyncInfo(
                                on_wait=extra[i:i + maxw], on_update=[]),
                        ))
                    inst.sync_info = mybir.SyncInfo(
                        on_wait=keep, on_update=list(si.on_update))
                newlist.append(inst)
            bb.instructions = newlist


def _get_nc():
    global _NC
    if _NC is None:
        _NC = _build_nc()
        # populate .instr bytes for extended InstISA (custom DVE ops);
        # raw Bass skips this codegen pass
        mybir.codegen_inst_isa_subclasses(_NC)
        split_wait_overflow(_NC)
    return _NC


def _to_bf16(a):
    return np.asarray(a, dtype=np.float32).astype(ml_dtypes.bfloat16)


def _bias_t(v, kt=KT):
    # [L, d] -> [L, 128, d//128] with column t = v[:, 128t:128t+128]
    v = np.asarray(v, dtype=np.float32)
    return np.ascontiguousarray(v.reshape(L, kt, 128).transpose(0, 2, 1))


def _fold(w, g, be):
    """LayerNorm fold: returns (w'', bias_delta) with
    x_hat @ w = (rstd*r) @ w'' + bias_delta  (per token rstd)."""
    wp = g[:, None] * w
    c = wp.sum(axis=0)
    return wp - c[None, :] / D, be @ w


def _tile_w(w, nchunk):
    """[L, D, n*512] -> [L, n, 128, KT, 512] matching the kernel's SBUF
    weight-tile layout so the DMA reads are contiguous."""
    Lw, Din, Dout = w.shape
    out = w.reshape(Lw, KT, 128, nchunk, 512).transpose(0, 3, 2, 1, 4)
    return np.ascontiguousarray(out)


def kernel(**inputs):
    nc = _get_nc()

    src = np.asarray(inputs["src"]).astype(np.int32).reshape(-1)      # [4096]
    src_mask = np.asarray(inputs["src_mask"]).astype(np.float32)      # [B,1,1,S]
    emb = np.asarray(inputs["emb"], dtype=np.float32)
    pe = np.asarray(inputs["pe"], dtype=np.float32)

    wq_f = np.asarray(inputs["wq"], dtype=np.float32).copy()
    wk_f = np.asarray(inputs["wk"], dtype=np.float32).copy()
    wv_f = np.asarray(inputs["wv"], dtype=np.float32).copy()
    wo_f = np.asarray(inputs["wo"], dtype=np.float32)
    w1_f = np.asarray(inputs["w1"], dtype=np.float32).copy()
    w2_f = np.asarray(inputs["w2"], dtype=np.float32)
    bq_f = np.asarray(inputs["bq"], dtype=np.float32).copy()
    bk_f = np.asarray(inputs["bk"], dtype=np.float32).copy()
    bv_f = np.asarray(inputs["bv"], dtype=np.float32).copy()
    bo_f = np.asarray(inputs["bo"], dtype=np.float32)
    b1_f = np.asarray(inputs["b1"], dtype=np.float32).copy()
    b2_f = np.asarray(inputs["b2"], dtype=np.float32)
    g1_f = np.asarray(inputs["g1"], dtype=np.float32)
    be1_f = np.asarray(inputs["be1"], dtype=np.float32)
    g2_f = np.asarray(inputs["g2"], dtype=np.float32)
    be2_f = np.asarray(inputs["be2"], dtype=np.float32)

    # fold LN1 into fc1 (all layers); fold LN2[l-1] into QKV[l] (l >= 1)
    for l in range(L):
        w1_f[l], d1 = _fold(w1_f[l], g1_f[l], be1_f[l])
        b1_f[l] = b1_f[l] + d1
        if l >= 1:
            g, be = g2_f[l - 1], be2_f[l - 1]
            wq_f[l], dq = _fold(wq_f[l], g, be)
            bq_f[l] = bq_f[l] + dq
            wk_f[l], dk_ = _fold(wk_f[l], g, be)
            bk_f[l] = bk_f[l] + dk_
            wv_f[l], dv = _fold(wv_f[l], g, be)
            bv_f[l] = bv_f[l] + dv
    # fold the V bias through the O projection: attn rows sum to 1, so
    # out = attn@(V + bv) @ wo + bo = attn@V@wo + (bv@wo + bo)
    bo_eff = np.stack([bo_f[l] + bv_f[l] @ wo_f[l] for l in range(L)])
    # O-proj PSUM seed: bo_eff plus the previous layer's LN2 shift (be2)
    bor_np = bo_eff.copy()
    for l in range(1, L):
        bor_np[l] = bor_np[l] + be2_f[l - 1]
    # fc2 PSUM seed: b2 plus this layer's LN1 shift (be1)
    b2r_np = b2_f + be1_f

    shared = {
        "emb": emb,
        "wq": _tile_w(_to_bf16(wq_f), 2), "wk": _tile_w(_to_bf16(wk_f), 2),
        "wv": _tile_w(_to_bf16(wv_f), 2), "wo": _tile_w(_to_bf16(wo_f), 2),
        "w1": _tile_w(_to_bf16(w1_f), 8), "w2": _to_bf16(w2_f),
        "bqT": _bias_t(bq_f), "bkT": _bias_t(bk_f),
        "b1T": _bias_t(b1_f, FT),
        "g1T": _bias_t(g1_f),
        "g2T": _bias_t(g2_f), "be2T": _bias_t(be2_f),
        "b2r": np.ascontiguousarray(_to_bf16(b2r_np).reshape(L, 1, D)),
        "bor": np.ascontiguousarray(_to_bf16(bor_np).reshape(L, 1, D)),
    }

    in_maps = []
    for c in range(NCORES):
        b = c // 2
        half = c % 2
        m = dict(shared)
        m["src"] = np.ascontiguousarray(
            src[c * TOK:(c + 1) * TOK].reshape(TOK, 1))
        m["peT"] = np.ascontiguousarray(
            pe[half * TOK:half * TOK + TOK, :D].T.astype(np.float32))
        mb = (src_mask[b, 0, 0, :] - 1.0) * 1e9
        own = slice(half * TOK, half * TOK + TOK)
        pair = slice((1 - half) * TOK, (1 - half) * TOK + TOK)
        mb_perm = np.concatenate([mb[own], mb[pair]])
        m["maskb"] = np.ascontiguousarray(
            mb_perm.reshape(KT, 128).T.astype(np.float32))
        o = 1 - half  # pair-local rank of the partner
        m["koidx"] = np.ascontiguousarray(
            (np.arange(512, dtype=np.int32) + o * 512).reshape(512, 1))
        # split V-AG layout: vag_out[hv] holds [rank0 rows, rank1 rows] of
        # 256-token slabs; partner token (mt*128+p) sits at o*256+(mt%2)*128+p
        vo = np.empty(TOK, dtype=np.int32)
        ar = np.arange(128, dtype=np.int32)
        for mt in range(4):
            vo[mt * 128:(mt + 1) * 128] = o * 256 + (mt % 2) * 128 + ar
        m["voidx"] = np.ascontiguousarray(vo.reshape(TOK, 1))
        in_maps.append(m)

    res = run_bass_kernel_spmd(nc, in_maps, list(range(NCORES)))
    out = np.empty((B * S, D), dtype=np.float32)
    for c in range(NCORES):
        out[c * TOK:(c + 1) * TOK] = res.results[c]["xout"].T
    return out.reshape(B, S, D)


# revision 50
# speedup vs baseline: 1.0719x; 1.0428x over previous
"""Trainium2 Bass kernel for a 6-layer transformer encoder (B=4, S=1024,
d_model=1024, 16 heads, d_ff=4096).

Sharding: token-parallel across 8 cores (B*S = 4096 tokens -> 512/core; each
core owns half of one batch element's sequence).  Per layer, pair-wise
AllGathers of K^T (bf16, 2 chunks) and V (fp8, vaug layout) let each core
rebuild K/V for its full batch element.

v5: the LayerNorm fold now pre-scales the residual once per layer
(rbn = rb * rstd, 8 cheap DVE bf16 muls) instead of applying the per-token
rstd at every PSUM drain.  All projection drains become single Scalar-engine
activations (Identity/Relu/Copy with per-partition bias), eliminating the
serialized 7.5us GpSimd tensor_scalar chain that dominated v4's critical
path.  The LN affine (g, be) is folded into the next projection: be rides a
K=1 ones-outer-product PSUM seed, g rides the drain's scalar_tensor_tensor.
Reciprocals use the ~5x faster approx-NR custom DVE op.  LN2's x-hat is kept
unscaled in the residual stream; the final layer applies g2/be2 on the way
out.
"""

import sys
import os

for _p in ("/opt/trn_rl_repo", "/root/.axon_site/_ro/trn_rl_repo"):
    if os.path.isdir(_p) and _p not in sys.path:
        sys.path.insert(0, _p)

import numpy as np
import ml_dtypes

import concourse.bass as bass
import concourse.mybir as mybir
import concourse.tile as tile
from concourse.bass_utils import run_bass_kernel_spmd
from concourse.masks import make_identity

VOCAB, D, H, DFF, L = 32000, 1024, 16, 4096, 6
B, S = 4, 1024
DK = D // H              # 64
NCORES = 8
TOK = (B * S) // NCORES  # 512 tokens per core
KT = D // 128            # 8
FT = DFF // 128          # 32
EPS = 1e-5
VS = 16.0                # fp8 V pre-scale (ones col carries the same scale)

F32 = mybir.dt.float32
BF16 = mybir.dt.bfloat16
FP8 = mybir.dt.float8e4
I32 = mybir.dt.int32
AF = mybir.ActivationFunctionType
OP = mybir.AluOpType
DR = mybir.MatmulPerfMode.DoubleRow

_NC = None
DEBUG = False

PAIRS = [[2 * i, 2 * i + 1] for i in range(NCORES // 2)]


def _build_nc():
    nc = bass.Bass("TRN2", target_bir_lowering=False, debug=False, num_devices=NCORES)

    emb = nc.dram_tensor("emb", [VOCAB, D], F32, kind="ExternalInput")
    src = nc.dram_tensor("src", [TOK, 1], I32, kind="ExternalInput")
    peT = nc.dram_tensor("peT", [D, TOK], F32, kind="ExternalInput")
    maskb = nc.dram_tensor("maskb", [128, KT], F32, kind="ExternalInput")
    koidx = nc.dram_tensor("koidx", [512, 1], I32, kind="ExternalInput")
    voidx = nc.dram_tensor("voidx", [TOK, 1], I32, kind="ExternalInput")
    # projection weights pre-tiled host-side to [L, chunk, 128, KT, cols]
    # so every weight DMA is a fully contiguous read
    wq = nc.dram_tensor("wq", [L, 2, 128, KT, 512], BF16, kind="ExternalInput")
    wk = nc.dram_tensor("wk", [L, 2, 128, KT, 512], BF16, kind="ExternalInput")
    wv = nc.dram_tensor("wv", [L, 2, 128, KT, 512], BF16, kind="ExternalInput")
    wo = nc.dram_tensor("wo", [L, 2, 128, KT, 512], BF16, kind="ExternalInput")
    w1 = nc.dram_tensor("w1", [L, 8, 128, KT, 512], BF16, kind="ExternalInput")
    w2 = nc.dram_tensor("w2", [L, DFF, D], BF16, kind="ExternalInput")
    bqT = nc.dram_tensor("bqT", [L, 128, KT], F32, kind="ExternalInput")
    bkT = nc.dram_tensor("bkT", [L, 128, KT], F32, kind="ExternalInput")
    b1T = nc.dram_tensor("b1T", [L, 128, FT], F32, kind="ExternalInput")
    g1T = nc.dram_tensor("g1T", [L, 128, KT], F32, kind="ExternalInput")
    g2T = nc.dram_tensor("g2T", [L, 128, KT], F32, kind="ExternalInput")
    be2T = nc.dram_tensor("be2T", [L, 128, KT], F32, kind="ExternalInput")
    # K=1 PSUM-seed rows: fc2 gets b2+be1, O-proj gets bo_eff(+be2[l-1])
    b2r = nc.dram_tensor("b2r", [L, 1, D], BF16, kind="ExternalInput")
    bor = nc.dram_tensor("bor", [L, 1, D], BF16, kind="ExternalInput")
    xout = nc.dram_tensor("xout", [D, TOK], F32, kind="ExternalOutput")
    dbg = {}
    if DEBUG:
        for nm, shp, dt in [
            ("dbg_x0", [128, KT, TOK], F32),     # post-embedding x
            ("dbg_ktl", [128, KT, TOK], BF16),   # K proj (own half)
            ("dbg_qt", [128, KT, TOK], BF16),    # Q proj
            ("dbg_attn", [128, KT, TOK], BF16),  # softmax @ V
            ("dbg_r1", [128, KT, TOK], F32),     # post O-drain residual
            ("dbg_ht", [128, FT, TOK], BF16),    # fc1 relu out
            ("dbg_r2", [128, KT, TOK], F32),     # post fc2 residual
            ("dbg_xh2", [128, KT, TOK], F32),    # xhat2 end of layer 0
        ]:
            dbg[nm] = nc.dram_tensor(nm, shp, dt, kind="ExternalOutput")

    from contextlib import ExitStack
    with tile.TileContext(nc) as tc:
        with ExitStack() as _es:
            def _pool(**kw):
                return _es.enter_context(tc.tile_pool(**kw))
            cpool = _pool(name="cpool", bufs=1)
            wp = _pool(name="wp", bufs=2)        # QKVO/fc1 weight chunks
            w2p = _pool(name="w2p", bufs=3)      # fc2 weight row-blocks
            pbig = _pool(name="pbig", bufs=1)    # ht
            ppe = _pool(name="ppe", bufs=1)      # peT (embedding only)
            px = _pool(name="px", bufs=1)        # f32 residual (in-place)
            pxb = _pool(name="pxb", bufs=2)      # bf16 pre-norm rb ping-pong
            pxn = _pool(name="pxn", bufs=2)      # bf16 rstd-scaled rbn
            pat = _pool(name="pat", bufs=1)      # ktl/kto/qt/attn
            pexp = _pool(name="pexp", bufs=2)    # exps per head-pair
            p2 = _pool(name="p2", bufs=2)        # transients (sq/stages)
            prs = _pool(name="prs", bufs=2)      # rstd rows
            pv3 = _pool(name="pv3", bufs=3)      # pavs (3-deep: 2-stage defer)
            bp = _pool(name="bp", bufs=8)        # bias rows
            psd = _pool(name="psd", bufs=1)      # K=1 PSUM-seed rows
            psc = _pool(name="psc", bufs=2, space="PSUM")   # 2-bank tiles
            ppv = _pool(name="ppv", bufs=2, space="PSUM")   # 1-bank tiles
            ppj = _pool(name="ppj", bufs=2, space="PSUM")   # proj psums
            dram = _pool(name="dram", bufs=2, space="DRAM")
            _uid = [0]

            def _nm(tag):
                _uid[0] += 1
                return f"{tag}_{_uid[0]}"

            ident = cpool.tile([128, 128], BF16, tag="ident", name=_nm("ident"))
            make_identity(nc, ident[:])
            onesk = cpool.tile([128, 128], BF16, tag="onesk", name=_nm("onesk"))
            nc.vector.memset(onesk[:], 1.0 / D)
            ones1t = cpool.tile([1, TOK], BF16, tag="ones1t", name=_nm("ones1t"))
            nc.vector.memset(ones1t[:], 1.0)
            maskb_sb = cpool.tile([128, KT], F32, tag="maskb", name=_nm("maskb"))
            nc.sync.dma_start(maskb_sb[:], maskb[:])
            eps_sb = cpool.tile([128, 1], F32, tag="eps", name=_nm("eps"))
            nc.vector.memset(eps_sb[:], EPS)
            # vaug: [tok-part, kt, head*65] fp8; col 64 of each head block is
            # the constant VS used to accumulate the softmax denominator.
            vaug = cpool.tile([128, KT, H * 65], FP8, tag="vaug", name=_nm("vaug"))
            nc.vector.memset(
                vaug[:].rearrange("p t (h x) -> p t h x", x=65)[:, :, :, 64:65], VS)

            def pj():
                return ppj.tile([128, 512], F32, tag="pj", name=_nm("pj"))

            def load_bias8(t, l):
                b = bp.tile([128, KT], F32, tag="bias8", name=_nm("bias8"))
                nc.sync.dma_start(b[:], t[l])
                return b

            def load_seed(t, l):
                s = psd.tile([1, D], BF16, tag="seedrow", name=_nm("seed"))
                nc.sync.dma_start(s[:], t[l])
                return s

            # ---------------- embedding ----------------
            peT_sb = ppe.tile([128, KT, TOK], F32, tag="peT", name=_nm("peT"))
            nc.sync.dma_start(peT_sb[:], peT.rearrange("(t p) n -> p t n", p=128))
            x_cur = px.tile([128, KT, TOK], F32, tag="x", name=_nm("x"))
            for blk in range(TOK // 128):
                idx_t = p2.tile([128, 1], I32, tag="idx", name=_nm("idx"))
                nc.sync.dma_start(idx_t[:], src[blk * 128:(blk + 1) * 128, :])
                gat = p2.tile([128, D], F32, tag="bfs", name=_nm("gat"))
                nc.gpsimd.indirect_dma_start(
                    out=gat[:], out_offset=None, in_=emb[:],
                    in_offset=bass.IndirectOffsetOnAxis(ap=idx_t[:, :1], axis=0),
                )
                gatb = p2.tile([128, D], BF16, tag="lns", name=_nm("gatb"))
                nc.scalar.activation(gatb[:], gat[:], AF.Copy)
                for kt in range(KT):
                    tp = ppj.tile([128, 512], BF16, tag="pj", name=_nm("tp"))
                    nc.tensor.transpose(tp[:, :128], gatb[:, kt * 128:(kt + 1) * 128],
                                        ident[:])
                    nc.vector.scalar_tensor_tensor(
                        out=x_cur[:, kt, blk * 128:(blk + 1) * 128],
                        in0=tp[:, :128], scalar=32.0,
                        in1=peT_sb[:, kt, blk * 128:(blk + 1) * 128],
                        op0=OP.mult, op1=OP.add,
                    )
            rbn_cur = pxn.tile([128, KT, TOK], BF16, tag="xn", name=_nm("xn0"))
            for kk in range(KT):
                nc.scalar.activation(rbn_cur[:, kk, :], x_cur[:, kk, :], AF.Copy)
            if DEBUG:
                nc.sync.dma_start(dbg["dbg_x0"][:], x_cur[:])

            # ------------- layer norm pieces -------------
            def ln_begin():
                st = psc.tile([128, 2, 512], F32, tag="sc", name=_nm("lnst"))
                rb = pxb.tile([128, KT, TOK], BF16, tag="xb", name=_nm("rb"))
                return {"st": st, "rb": rb}

            def ln_stat(s, r, mg):
                nc.scalar.activation(s["rb"][:, mg, :], r[:, mg, :], AF.Copy)
                sq = p2.tile([128, TOK], BF16, tag="sq1", name=_nm("sq"))
                nc.vector.tensor_mul(sq[:], s["rb"][:, mg, :],
                                     s["rb"][:, mg, :])
                nc.tensor.matmul(s["st"][:, 0, :], onesk[:], s["rb"][:, mg, :],
                                 start=(mg == 0), stop=(mg == KT - 1))
                nc.tensor.matmul(s["st"][:, 1, :], onesk[:], sq[:],
                                 start=(mg == 0), stop=(mg == KT - 1))

            def ln_final(s):
                # rstd = sqrt(1/(var+eps)): the approx-reciprocal runs on the
                # raw variance so both the f32 and bf16 sqrt taps come straight
                # off one DVE chain (shorter than sqrt->recip->cast)
                msq = p2.tile([128, TOK], F32, tag="lns", name=_nm("msq"))
                nc.scalar.activation(msq[:], s["st"][:, 0, :], AF.Square)
                inv = p2.tile([128, TOK], F32, tag="lns", name=_nm("inv"))
                nc.vector.scalar_tensor_tensor(
                    out=inv[:], in0=s["st"][:, 1, :], scalar=eps_sb[:, 0:1],
                    in1=msq[:], op0=OP.add, op1=OP.subtract)
                nc.vector.reciprocal_approx_fast(inv[:], inv[:])
                rstd = prs.tile([128, TOK], F32, tag="rstd", name=_nm("rstd"))
                nc.scalar.activation(rstd[:], inv[:], AF.Sqrt)
                rs16 = prs.tile([128, TOK], BF16, tag="rs16", name=_nm("rs16"))
                nc.scalar.activation(rs16[:], inv[:], AF.Sqrt)
                s["rstd"] = rstd
                s["rs16"] = rs16

            def ln_xhat_kk(s, r, kk):
                # in-place normalize of the residual stream (r -> x-hat,
                # WITHOUT the g/be affine - that is folded downstream)
                nc.vector.tensor_sub(r[:, kk, :], r[:, kk, :], s["st"][:, 0, :])
                nc.vector.tensor_mul(r[:, kk, :], r[:, kk, :], s["rstd"][:])

            # ---------------- layers ----------------
            wkh_pre = [None, None]
            for l in range(L):
                bk_sb = load_bias8(bkT, l)
                bq_sb = load_bias8(bqT, l)
                bor_sb = load_seed(bor, l)
                rbn_in = rbn_cur        # rstd-scaled residual (or embedding)

                # --- K projection + chunked pair-AllGather of K^T ---
                # weight loads ride the ACT queue (nc.scalar) one phase ahead
                # of their consumers so the PE never waits on LDWEIGHTS input
                def wload(t, src):
                    w = wp.tile([128, KT, 512], BF16, tag="wproj", name=_nm(t))
                    nc.scalar.dma_start(w[:], src)
                    return w

                # first layer loads its K weights here; later layers get them
                # prefetched during the previous layer's fc2
                wkh_t = wkh_pre if l > 0 else [wload("wk", wk[l, h])
                                               for h in range(2)]
                wvh_t = [None, None]
                wq_tiles = {}
                ktl = pat.tile([128, KT, TOK], BF16, tag="ktl", name=_nm("ktl"))
                kag_in = dram.tile([D, TOK], BF16, tag="kag_in", name=_nm("kag_in"))
                kag_out = [
                    dram.tile([2 * 512, TOK], BF16, tag=f"kag_out{c}",
                              name=_nm("kag_out")) for c in range(2)]
                kto = pat.tile([128, KT, TOK], BF16, tag="kto", name=_nm("kto"))
                for half in range(2):
                    wkh = wkh_t[half]
                    for m in range(4):
                        mg = half * 4 + m
                        pt = pj()
                        for kk in range(KT):
                            nc.tensor.matmul(
                                pt[:], wkh[:, kk, m * 128:(m + 1) * 128],
                                rbn_in[:, kk, :],
                                start=(kk == 0), stop=(kk == KT - 1))
                        nc.scalar.activation(ktl[:, mg, :], pt[:], AF.Identity,
                                             bias=bk_sb[:, mg:mg + 1])
                        nc.sync.dma_start(
                            kag_in[mg * 128:(mg + 1) * 128, :], ktl[:, mg, :])
                    nc.gpsimd.collective_compute(
                        "AllGather", OP.bypass,
                        ins=[kag_in[half * 512:(half + 1) * 512, :]],
                        outs=[kag_out[half][:]],
                        replica_groups=PAIRS,
                    )
                    wvh_t[half] = wload("wv", wv[l, half])
                    # partner-K gathers ride right behind this half's AG
                    for g in range(4 * half, 4 * half + 4):
                        kidx = bp.tile([128, 1], I32, tag="koidx",
                                       name=_nm("koidx"))
                        nc.sync.dma_start(
                            kidx[:], koidx[(g % 4) * 128:(g % 4) * 128 + 128, :])
                        nc.gpsimd.indirect_dma_start(
                            out=kto[:, g, :], out_offset=None,
                            in_=kag_out[g // 4][:],
                            in_offset=bass.IndirectOffsetOnAxis(
                                ap=kidx[:, :1], axis=0),
                        )
                if DEBUG and l == 0:
                    nc.sync.dma_start(dbg["dbg_ktl"][:], ktl[:])

                # --- V projection into vaug (own slots) + fp8 AllGather ---
                # mt-outer so each 128-token row block is complete (both head
                # halves) early; the AG is split in two so partner-V gathers
                # start at the V-phase midpoint instead of after it
                vag_in = dram.tile([TOK, H * 65], FP8, tag="vag_in", name=_nm("vag_in"))
                vag_out = [
                    dram.tile([2 * 256, H * 65], FP8, tag=f"vag_out{c}",
                              name=_nm("vag_out")) for c in range(2)]
                for mt in range(4):
                    for half in range(2):
                        wvh = wvh_t[half]
                        pt = pj()
                        for kk in range(KT):
                            nc.tensor.matmul(
                                pt[:], rbn_in[:, kk, mt * 128:(mt + 1) * 128],
                                wvh[:, kk, :],
                                start=(kk == 0), stop=(kk == KT - 1))
                        vdst = vaug[:, mt, :].rearrange(
                            "p (h x) -> p h x", x=65)[:, half * 8:(half + 1) * 8, 0:64]
                        vsrc = pt[:].rearrange("p (h c) -> p h c", c=64)
                        nc.scalar.activation(vdst, vsrc, AF.Copy, scale=VS)
                    nc.sync.dma_start(
                        vag_in[mt * 128:(mt + 1) * 128, :], vaug[:, mt, :])
                    if mt % 2 == 1:
                        hv = mt // 2
                        nc.gpsimd.collective_compute(
                            "AllGather", OP.bypass,
                            ins=[vag_in[hv * 256:(hv + 1) * 256, :]],
                            outs=[vag_out[hv][:]],
                            replica_groups=PAIRS,
                        )
                        if hv == 0:
                            wq_tiles[0] = wload("wq", wq[l, 0])
                        for mt2 in (2 * hv, 2 * hv + 1):
                            vidx = bp.tile([128, 1], I32, tag="voidx",
                                           name=_nm("voidx"))
                            nc.sync.dma_start(
                                vidx[:], voidx[mt2 * 128:(mt2 + 1) * 128, :])
                            nc.gpsimd.indirect_dma_start(
                                out=vaug[:, 4 + mt2, :], out_offset=None,
                                in_=vag_out[hv][:],
                                in_offset=bass.IndirectOffsetOnAxis(
                                    ap=vidx[:, :1], axis=0),
                            )
                wq_tiles[1] = wload("wq", wq[l, 1])

                # --- Q projection woven with attention at 2-MM granularity ---
                qt = pat.tile([128, KT, TOK], BF16, tag="qt", name=_nm("qt"))
                # per-m-tile attn tiles: O-proj's kk-th matmul depends only on
                # tile kk, so the tail heads' softmax chains overlap O-proj
                attn_t = [pat.tile([128, TOK], BF16, tag=f"attn{j}",
                                   name=_nm("attn")) for j in range(KT)]
                recips_d = dram.tile([H, TOK], F32, tag="recips_d", name=_nm("recd"))
                recips_r = dram.tile([H, TOK], F32, tag="recips_r", name=_nm("recr"))
                exps_tiles = {}

                def kt_lhs(kt, mj, prow):
                    if kt < 4:
                        return ktl[prow:prow + 64, mj, kt * 128:(kt + 1) * 128]
                    return kto[prow:prow + 64, mj, (kt - 4) * 128:(kt - 3) * 128]

                def s_unit(mj, kt):
                    # one kt-block of scores for BOTH heads of m-tile mj; a
                    # single exp instruction covers the pair (same mask col)
                    if kt == 0:
                        exps_tiles[mj] = pexp.tile([128, KT, 2, TOK], FP8,
                                                   tag="exps", name=_nm("ex"))
                    ex = exps_tiles[mj]
                    st = psc.tile([128, 2, 512], F32, tag="sc", name=_nm("sc"))
                    for j in range(2):
                        prow = j * 64
                        nc.tensor.matmul(
                            st[:, j, :], kt_lhs(kt, mj, prow),
                            qt[prow:prow + 64, mj, :],
                            start=True, stop=True)
                    nc.scalar.activation(
                        ex[:, kt, :, :], st[:, :, :], AF.Exp,
                        scale=DK ** -0.5, bias=maskb_sb[:, kt:kt + 1])

                av_state = {}
                av_p1 = {}
                av_p2 = {}

                def av_stage1(h):
                    # one head-slot after the dT reload was issued: the recip
                    # never head-of-line-blocks the DVE queue on that DMA
                    mj, pavs, dT = av_p1.pop(h)
                    nc.vector.reciprocal_approx_fast(dT[:], dT[:])
                    nc.gpsimd.dma_start(
                        recips_r[h:h + 1, :].rearrange(
                            "o (p f) -> (o p) f", p=128), dT[:])
                    rbc = p2.tile([64, TOK], F32, tag="lns", name=_nm("rbc"))
                    nc.sync.dma_start(
                        rbc[:],
                        recips_r[h:h + 1, None, :].to_broadcast((1, 64, TOK)))
                    av_p2[h] = (mj, pavs, rbc)

                def av_stage2(h):
                    # and the attn mul one further slot later, once rbc landed
                    mj, pavs, rbc = av_p2.pop(h)
                    if h % 2 == 0:
                        nc.vector.tensor_mul(
                            attn_t[mj][0:64, :], pavs[0:64, :], rbc[:])
                    else:
                        stg = p2.tile([64, TOK], BF16, tag="stage",
                                      name=_nm("stg"))
                        nc.vector.tensor_mul(stg[:], pavs[0:64, :], rbc[:])
                        nc.sync.dma_start(attn_t[mj][64:128, :], stg[:])

                def av_unit(h, c):
                    mj = h // 2
                    ex = exps_tiles[mj]
                    if c == 0:
                        av_state[h] = ppv.tile([128, 512], F32, tag="pav",
                                               name=_nm("pav"))
                    pav = av_state[h]
                    vo = 65 * h
                    hs = h % 2
                    for p in (2 * c, 2 * c + 1):
                        nc.tensor.matmul(
                            pav[0:65, :], vaug[:, 2 * p:2 * p + 2, vo:vo + 65],
                            ex[:, 2 * p:2 * p + 2, hs, :],
                            start=(p == 0), stop=(p == 3),
                            perf_mode=DR)
                    if c == 1:
                        if hs == 1:
                            exps_tiles.pop(mj)
                        av_state.pop(h)
                        # free the PSUM bank fast; the rest of the softmax
                        # chain lags 1-2 head-slots behind the SBUF copy
                        pavs = pv3.tile([65, TOK], F32, tag="pavs", name=_nm("pavs"))
                        nc.vector.tensor_copy(pavs[:], pav[0:65, :])
                        nc.sync.dma_start(recips_d[h:h + 1, :], pavs[64:65, :])
                        # approx reciprocal is only valid on full-partition
                        # tiles: round-trip the denom row through DRAM as
                        # [128, 4] (130ns) instead of a 3.3us 1-lane recip
                        dT = bp.tile([128, 4], F32, tag="dT", name=_nm("dT"))
                        nc.gpsimd.dma_start(
                            dT[:], recips_d[h:h + 1, :].rearrange(
                                "o (p f) -> (o p) f", p=128))
                        av_p1[h] = (mj, pavs, dT)
                        if (h - 1) in av_p1:
                            av_stage1(h - 1)
                        if (h - 2) in av_p2:
                            av_stage2(h - 2)

                qpt = {}

                def q_unit(mg, c):
                    if c == 0:
                        qpt[mg] = pj()
                    wqh = wq_tiles[mg // 4]
                    m = mg % 4
                    for kk in (2 * c, 2 * c + 1):
                        nc.tensor.matmul(
                            qpt[mg][:], wqh[:, kk, m * 128:(m + 1) * 128],
                            rbn_in[:, kk, :],
                            start=(kk == 0), stop=(kk == KT - 1))
                    if c == 3:
                        nc.vector.tensor_scalar_add(
                            qt[:, mg, :], qpt.pop(mg)[:], bq_sb[:, mg:mg + 1])

                woh_t = [None, None]
                for c in range(4):
                    q_unit(0, c)
                for mg in range(KT):
                    if mg == 5:
                        woh_t[0] = wload("wo", wo[l, 0])
                    if mg == 7:
                        woh_t[1] = wload("wo", wo[l, 1])
                    units = []
                    if mg >= 1:
                        h0 = 2 * (mg - 1)
                        units += [("av", h0, 0), ("av", h0, 1),
                                  ("av", h0 + 1, 0), ("av", h0 + 1, 1)]
                    if mg + 1 < KT:
                        units += [("q", mg + 1, c) for c in range(4)]
                    s_units = [("s", mg, kt) for kt in range(KT)]
                    woven = []
                    oi = 0
                    for su in s_units:
                        woven.append(su)
                        if oi < len(units):
                            woven.append(units[oi]); oi += 1
                    woven += units[oi:]
                    for u in woven:
                        if u[0] == "s":
                            s_unit(u[1], u[2])
                        elif u[0] == "q":
                            q_unit(u[1], u[2])
                        else:
                            av_unit(u[1], u[2])
                for h in (14, 15):
                    for c in range(2):
                        av_unit(h, c)
                av_stage1(15)
                av_stage2(14)
                av_stage2(15)
                if DEBUG and l == 0:
                    nc.sync.dma_start(dbg["dbg_qt"][:], qt[:])
                    for j in range(KT):
                        nc.sync.dma_start(dbg["dbg_attn"][:, j, :], attn_t[j][:])

                # --- O projection + residual; LN1 stats woven per m-tile ---
                # PSUM is seeded with bo_eff (+be2[l-1]); for l>=1 the drain
                # applies g2[l-1] to the x-hat residual stream.
                g1_sb = load_bias8(g1T, l)
                g2p_sb = load_bias8(g2T, l - 1) if l >= 1 else None
                # kk-outer over 4 accumulators, two passes (mg 0-3 on psc with
                # woh0, mg 4-7 on ppv+ppj with woh1): the attention tail's
                # softmax chains overlap the O matmuls, and woh0 dies at the
                # pass boundary so the first fc1 weight chunk loads early
                o_a = psc.tile([128, 2, 512], F32, tag="sc", name=_nm("oa"))
                o_a2 = psc.tile([128, 2, 512], F32, tag="sc", name=_nm("oa2"))
                o_b = ppv.tile([128, 512], F32, tag="pav", name=_nm("ob"))
                o_c = ppv.tile([128, 512], F32, tag="pav", name=_nm("oc"))
                o_d = pj()
                o_e = pj()
                ops = [o_a[:, 0, :], o_a[:, 1, :], o_a2[:, 0, :], o_a2[:, 1, :],
                       o_b[:], o_c[:], o_d[:], o_e[:]]
                for half in range(2):
                    mgs = range(4 * half, 4 * half + 4)
                    for mg in mgs:
                        nc.tensor.matmul(
                            ops[mg], bor_sb[0:1, mg * 128:(mg + 1) * 128],
                            ones1t[0:1, :], start=True, stop=False)
                    for kk in range(KT):
                        for mg in mgs:
                            nc.tensor.matmul(
                                ops[mg],
                                woh_t[half][:, kk, (mg % 4) * 128:(mg % 4 + 1) * 128],
                                attn_t[kk][:],
                                start=False, stop=(kk == KT - 1))
                    if half == 0:
                        w1e_next = [wload("w1e", w1[l, 0])]

                def o_drain(mg):
                    if l == 0:
                        nc.vector.tensor_add(
                            x_cur[:, mg, :], ops[mg], x_cur[:, mg, :])
                    else:
                        nc.vector.scalar_tensor_tensor(
                            out=x_cur[:, mg, :], in0=x_cur[:, mg, :],
                            scalar=g2p_sb[:, mg:mg + 1], in1=ops[mg],
                            op0=OP.mult, op1=OP.add)

                # drains 0-1 must precede ln_begin (its stats tile takes o_a's
                # PSUM banks); the rest interleave with the stats
                o_drain(0)
                o_drain(1)
                ln1 = ln_begin()
                for mg in range(2, KT):
                    o_drain(mg)
                    ln_stat(ln1, x_cur, mg - 2)
                ln_stat(ln1, x_cur, 6)
                ln_stat(ln1, x_cur, 7)
                ln_final(ln1)
                if DEBUG and l == 0:
                    nc.sync.dma_start(dbg["dbg_r1"][:], x_cur[:])
                rb1 = ln1["rb"]
                # rbn1 = rb1 * rstd1 : fc1's folded weights consume this
                rbn1 = pxn.tile([128, KT, TOK], BF16, tag="xn", name=_nm("rbn1"))
                for kk in range(KT):
                    nc.vector.tensor_mul(rbn1[:, kk, :], rb1[:, kk, :],
                                         ln1["rs16"][:])

                # --- fc1 on the rstd-scaled residual (folded weights) ---
                b1_sb = bp.tile([128, FT], F32, tag="bias32", name=_nm("b1"))
                nc.sync.dma_start(b1_sb[:], b1T[l])
                b2r_sb = load_seed(b2r, l)
                g2_sb = load_bias8(g2T, l)
                ht = pbig.tile([128, FT, TOK], BF16, tag="big32", name=_nm("ht"))
                for e in range(8):
                    w1e = w1e_next[0]
                    if e < 7:
                        w1e_next[0] = wload("w1e", w1[l, e + 1])
                    for m in range(4):
                        fm = e * 4 + m
                        # 4-deep psum rotation (pj + ppv) keeps the PE fed
                        if fm % 2 == 0:
                            pt = pj()
                        else:
                            pt = ppv.tile([128, 512], F32, tag="pav",
                                          name=_nm("fpv"))
                        for kk in range(KT):
                            nc.tensor.matmul(
                                pt[:], w1e[:, kk, m * 128:(m + 1) * 128],
                                rbn1[:, kk, :],
                                start=(kk == 0), stop=(kk == KT - 1))
                        nc.scalar.activation(ht[:, fm, :], pt[:], AF.Relu,
                                             bias=b1_sb[:, fm:fm + 1])
                if DEBUG and l == 0:
                    nc.sync.dma_start(dbg["dbg_ht"][:], ht[:])
                # normalize the residual in place (r1 -> xhat1; g1/be1 are
                # folded into the fc2 drain/seed)
                for kk in range(KT):
                    ln_xhat_kk(ln1, x_cur, kk)

                # --- fc2 kk-outer over 8 accumulators; contiguous w2 loads ---
                f_a = psc.tile([128, 2, 512], F32, tag="sc", name=_nm("fa"))
                f_a2 = psc.tile([128, 2, 512], F32, tag="sc", name=_nm("fa2"))
                f_b = ppv.tile([128, 512], F32, tag="pav", name=_nm("fb"))
                f_c = ppv.tile([128, 512], F32, tag="pav", name=_nm("fc"))
                f_d = pj()
                f_e = pj()
                fps = [f_a[:, 0, :], f_a[:, 1, :], f_a2[:, 0, :], f_a2[:, 1, :],
                       f_b[:], f_c[:], f_d[:], f_e[:]]
                for mg in range(KT):
                    nc.tensor.matmul(
                        fps[mg], b2r_sb[0:1, mg * 128:(mg + 1) * 128],
                        ones1t[0:1, :], start=True, stop=False)

                # w2 row-blocks: split each load across both HWDGE queues and
                # run the prefetch two chunks deep so the PE never starves
                def w2load(kk):
                    t = w2p.tile([128, 1024], BF16, tag="w2c", name=_nm("w2c"))
                    nc.sync.dma_start(
                        t[:, 0:512], w2[l, kk * 128:(kk + 1) * 128, 0:512])
                    nc.scalar.dma_start(
                        t[:, 512:1024], w2[l, kk * 128:(kk + 1) * 128, 512:1024])
                    return t

                w2q = [w2load(0), w2load(1)]
                for kk in range(FT):
                    if kk == 20 and l + 1 < L:
                        wkh_pre = [wload("wk", wk[l + 1, h]) for h in range(2)]
                    w2c = w2q.pop(0)
                    if kk < FT - 2:
                        w2q.append(w2load(kk + 2))
                    for mg in range(KT):
                        nc.tensor.matmul(
                            fps[mg], w2c[:, mg * 128:(mg + 1) * 128],
                            ht[:, kk, :],
                            start=False, stop=(kk == FT - 1))
                def f_drain(mg):
                    # r2 = xhat1*g1 + (h@w2 + b2 + be1)   (seeded PSUM)
                    nc.vector.scalar_tensor_tensor(
                        out=x_cur[:, mg, :], in0=x_cur[:, mg, :],
                        scalar=g1_sb[:, mg:mg + 1], in1=fps[mg],
                        op0=OP.mult, op1=OP.add)

                # drains 0-1 free f_a's banks before ln_begin claims them;
                # the remaining drains interleave with the stats
                f_drain(0)
                f_drain(1)
                ln2 = ln_begin()
                for mg in range(2, KT):
                    f_drain(mg)
                    ln_stat(ln2, x_cur, mg - 2)
                ln_stat(ln2, x_cur, 6)
                ln_stat(ln2, x_cur, 7)
                if DEBUG and l == 0:
                    nc.sync.dma_start(dbg["dbg_r2"][:], x_cur[:])
                ln_final(ln2)
                rb2 = ln2["rb"]
                if l < L - 1:
                    rbn_cur = pxn.tile([128, KT, TOK], BF16, tag="xn",
                                       name=_nm("rbn2"))
                    for kk in range(KT):
                        nc.vector.tensor_mul(rbn_cur[:, kk, :], rb2[:, kk, :],
                                             ln2["rs16"][:])
                    for kk in range(KT):
                        ln_xhat_kk(ln2, x_cur, kk)
                    if DEBUG and l == 0:
                        nc.sync.dma_start(dbg["dbg_xh2"][:], x_cur[:])
                else:
                    be2_sb = load_bias8(be2T, l)
                    for kk in range(KT):
                        ln_xhat_kk(ln2, x_cur, kk)
                        nc.scalar.activation(
                            x_cur[:, kk, :], x_cur[:, kk, :], AF.Identity,
                            bias=be2_sb[:, kk:kk + 1], scale=g2_sb[:, kk:kk + 1])

            nc.sync.dma_start(
                xout.rearrange("(t p) n -> p t n", p=128), x_cur[:])

    return nc


MAXW = 1


def split_wait_overflow(nc, maxw=MAXW):
    """walrus in this toolchain rejects instructions with more than one sem
    wait; split excess waits onto preceding NoOp carriers on the same engine."""
    for f in nc.m.functions:
        for bb in f.blocks:
            if not any(i.sync_info and len(i.sync_info.on_wait) > maxw
                       for i in bb.instructions):
                continue
            newlist = []
            for inst in bb.instructions:
                si = inst.sync_info
                if si and len(si.on_wait) > maxw:
                    waits = list(si.on_wait)
                    extra, keep = waits[:-maxw], waits[-maxw:]
                    for i in range(0, len(extra), maxw):
                        newlist.append(mybir.InstNoOp(
                            name=f"{inst.name}-ws{i}", opcode="NoOp",
                            engine=inst.engine, debug=inst.debug, ins=[], outs=[],
                            sync_info=mybir.SyncInfo(
                                on_wait=extra[i:i + maxw], on_update=[]),
                        ))
                    inst.sync_info = mybir.SyncInfo(
                        on_wait=keep, on_update=list(si.on_update))
                newlist.append(inst)
            bb.instructions = newlist


def _get_nc():
    global _NC
    if _NC is None:
        _NC = _build_nc()
        # populate .instr bytes for extended InstISA (custom DVE ops);
        # raw Bass skips this codegen pass
        mybir.codegen_inst_isa_subclasses(_NC)
        split_wait_overflow(_NC)
    return _NC


def _to_bf16(a):
    return np.asarray(a, dtype=np.float32).astype(ml_dtypes.bfloat16)


def _bias_t(v, kt=KT):
    # [L, d] -> [L, 128, d//128] with column t = v[:, 128t:128t+128]
    v = np.asarray(v, dtype=np.float32)
    return np.ascontiguousarray(v.reshape(L, kt, 128).transpose(0, 2, 1))


def _fold(w, g, be):
    """LayerNorm fold: returns (w'', bias_delta) with
    x_hat @ w = (rstd*r) @ w'' + bias_delta  (per token rstd)."""
    wp = g[:, None] * w
    c = wp.sum(axis=0)
    return wp - c[None, :] / D, be @ w


def _tile_w(w, nchunk):
    """[L, D, n*512] -> [L, n, 128, KT, 512] matching the kernel's SBUF
    weight-tile layout so the DMA reads are contiguous."""
    Lw, Din, Dout = w.shape
    out = w.reshape(Lw, KT, 128, nchunk, 512).transpose(0, 3, 2, 1, 4)
    return np.ascontiguousarray(out)


def kernel(**inputs):
    nc = _get_nc()

    src = np.asarray(inputs["src"]).astype(np.int32).reshape(-1)      # [4096]
    src_mask = np.asarray(inputs["src_mask"]).astype(np.float32)      # [B,1,1,S]
    emb = np.asarray(inputs["emb"], dtype=np.float32)
    pe = np.asarray(inputs["pe"], dtype=np.float32)

    wq_f = np.asarray(inputs["wq"], dtype=np.float32).copy()
    wk_f = np.asarray(inputs["wk"], dtype=np.float32).copy()
    wv_f = np.asarray(inputs["wv"], dtype=np.float32).copy()
    wo_f = np.asarray(inputs["wo"], dtype=np.float32)
    w1_f = np.asarray(inputs["w1"], dtype=np.float32).copy()
    w2_f = np.asarray(inputs["w2"], dtype=np.float32)
    bq_f = np.asarray(inputs["bq"], dtype=np.float32).copy()
    bk_f = np.asarray(inputs["bk"], dtype=np.float32).copy()
    bv_f = np.asarray(inputs["bv"], dtype=np.float32).copy()
    bo_f = np.asarray(inputs["bo"], dtype=np.float32)
    b1_f = np.asarray(inputs["b1"], dtype=np.float32).copy()
    b2_f = np.asarray(inputs["b2"], dtype=np.float32)
    g1_f = np.asarray(inputs["g1"], dtype=np.float32)
    be1_f = np.asarray(inputs["be1"], dtype=np.float32)
    g2_f = np.asarray(inputs["g2"], dtype=np.float32)
    be2_f = np.asarray(inputs["be2"], dtype=np.float32)

    # fold LN1 into fc1 (all layers); fold LN2[l-1] into QKV[l] (l >= 1)
    for l in range(L):
        w1_f[l], d1 = _fold(w1_f[l], g1_f[l], be1_f[l])
        b1_f[l] = b1_f[l] + d1
        if l >= 1:
            g, be = g2_f[l - 1], be2_f[l - 1]
            wq_f[l], dq = _fold(wq_f[l], g, be)
            bq_f[l] = bq_f[l] + dq
            wk_f[l], dk_ = _fold(wk_f[l], g, be)
            bk_f[l] = bk_f[l] + dk_
            wv_f[l], dv = _fold(wv_f[l], g, be)
            bv_f[l] = bv_f[l] + dv
    # fold the V bias through the O projection: attn rows sum to 1, so
    # out = attn@(V + bv) @ wo + bo = attn@V@wo + (bv@wo + bo)
    bo_eff = np.stack([bo_f[l] + bv_f[l] @ wo_f[l] for l in range(L)])
    # O-proj PSUM seed: bo_eff plus the previous layer's LN2 shift (be2)
    bor_np = bo_eff.copy()
    for l in range(1, L):
        bor_np[l] = bor_np[l] + be2_f[l - 1]
    # fc2 PSUM seed: b2 plus this layer's LN1 shift (be1)
    b2r_np = b2_f + be1_f

    shared = {
        "emb": emb,
        "wq": _tile_w(_to_bf16(wq_f), 2), "wk": _tile_w(_to_bf16(wk_f), 2),
        "wv": _tile_w(_to_bf16(wv_f), 2), "wo": _tile_w(_to_bf16(wo_f), 2),
        "w1": _tile_w(_to_bf16(w1_f), 8), "w2": _to_bf16(w2_f),
        "bqT": _bias_t(bq_f), "bkT": _bias_t(bk_f),
        "b1T": _bias_t(b1_f, FT),
        "g1T": _bias_t(g1_f),
        "g2T": _bias_t(g2_f), "be2T": _bias_t(be2_f),
        "b2r": np.ascontiguousarray(_to_bf16(b2r_np).reshape(L, 1, D)),
        "bor": np.ascontiguousarray(_to_bf16(bor_np).reshape(L, 1, D)),
    }

    in_maps = []
    for c in range(NCORES):
        b = c // 2
        half = c % 2
        m = dict(shared)
        m["src"] = np.ascontiguousarray(
            src[c * TOK:(c + 1) * TOK].reshape(TOK, 1))
        m["peT"] = np.ascontiguousarray(
            pe[half * TOK:half * TOK + TOK, :D].T.astype(np.float32))
        mb = (src_mask[b, 0, 0, :] - 1.0) * 1e9
        own = slice(half * TOK, half * TOK + TOK)
        pair = slice((1 - half) * TOK, (1 - half) * TOK + TOK)
        mb_perm = np.concatenate([mb[own], mb[pair]])
        m["maskb"] = np.ascontiguousarray(
            mb_perm.reshape(KT, 128).T.astype(np.float32))
        o = 1 - half  # pair-local rank of the partner
        m["koidx"] = np.ascontiguousarray(
            (np.arange(512, dtype=np.int32) + o * 512).reshape(512, 1))
        # split V-AG layout: vag_out[hv] holds [rank0 rows, rank1 rows] of
        # 256-token slabs; partner token (mt*128+p) sits at o*256+(mt%2)*128+p
        vo = np.empty(TOK, dtype=np.int32)
        ar = np.arange(128, dtype=np.int32)
        for mt in range(4):
            vo[mt * 128:(mt + 1) * 128] = o * 256 + (mt % 2) * 128 + ar
        m["voidx"] = np.ascontiguousarray(vo.reshape(TOK, 1))
        in_maps.append(m)

    res = run_bass_kernel_spmd(nc, in_maps, list(range(NCORES)))
    out = np.empty((B * S, D), dtype=np.float32)
    for c in range(NCORES):
        out[c * TOK:(c + 1) * TOK] = res.results[c]["xout"].T
    return out.reshape(B, S, D)


# revision 51
# speedup vs baseline: 1.0785x; 1.0061x over previous
"""Trainium2 Bass kernel for a 6-layer transformer encoder (B=4, S=1024,
d_model=1024, 16 heads, d_ff=4096).

Sharding: token-parallel across 8 cores (B*S = 4096 tokens -> 512/core; each
core owns half of one batch element's sequence).  Per layer, pair-wise
AllGathers of K^T (bf16, 2 chunks) and V (fp8, vaug layout) let each core
rebuild K/V for its full batch element.

v5: the LayerNorm fold now pre-scales the residual once per layer
(rbn = rb * rstd, 8 cheap DVE bf16 muls) instead of applying the per-token
rstd at every PSUM drain.  All projection drains become single Scalar-engine
activations (Identity/Relu/Copy with per-partition bias), eliminating the
serialized 7.5us GpSimd tensor_scalar chain that dominated v4's critical
path.  The LN affine (g, be) is folded into the next projection: be rides a
K=1 ones-outer-product PSUM seed, g rides the drain's scalar_tensor_tensor.
Reciprocals use the ~5x faster approx-NR custom DVE op.  LN2's x-hat is kept
unscaled in the residual stream; the final layer applies g2/be2 on the way
out.
"""

import sys
import os

for _p in ("/opt/trn_rl_repo", "/root/.axon_site/_ro/trn_rl_repo"):
    if os.path.isdir(_p) and _p not in sys.path:
        sys.path.insert(0, _p)

import numpy as np
import ml_dtypes

import concourse.bass as bass
import concourse.mybir as mybir
import concourse.tile as tile
from concourse.bass_utils import run_bass_kernel_spmd
from concourse.masks import make_identity

VOCAB, D, H, DFF, L = 32000, 1024, 16, 4096, 6
B, S = 4, 1024
DK = D // H              # 64
NCORES = 8
TOK = (B * S) // NCORES  # 512 tokens per core
KT = D // 128            # 8
FT = DFF // 128          # 32
EPS = 1e-5
VS = 16.0                # fp8 V pre-scale (ones col carries the same scale)

F32 = mybir.dt.float32
BF16 = mybir.dt.bfloat16
FP8 = mybir.dt.float8e4
I32 = mybir.dt.int32
AF = mybir.ActivationFunctionType
OP = mybir.AluOpType
DR = mybir.MatmulPerfMode.DoubleRow

_NC = None
DEBUG = False

PAIRS = [[2 * i, 2 * i + 1] for i in range(NCORES // 2)]


def _build_nc():
    nc = bass.Bass("TRN2", target_bir_lowering=False, debug=False, num_devices=NCORES)

    emb = nc.dram_tensor("emb", [VOCAB, D], F32, kind="ExternalInput")
    src = nc.dram_tensor("src", [TOK, 1], I32, kind="ExternalInput")
    peT = nc.dram_tensor("peT", [D, TOK], F32, kind="ExternalInput")
    maskb = nc.dram_tensor("maskb", [128, KT], F32, kind="ExternalInput")
    koidx = nc.dram_tensor("koidx", [512, 1], I32, kind="ExternalInput")
    voidx = nc.dram_tensor("voidx", [TOK, 1], I32, kind="ExternalInput")
    # projection weights pre-tiled host-side to [L, chunk, 128, KT, cols]
    # so every weight DMA is a fully contiguous read
    wq = nc.dram_tensor("wq", [L, 2, 128, KT, 512], BF16, kind="ExternalInput")
    wk = nc.dram_tensor("wk", [L, 2, 128, KT, 512], BF16, kind="ExternalInput")
    wv = nc.dram_tensor("wv", [L, 2, 128, KT, 512], BF16, kind="ExternalInput")
    wo = nc.dram_tensor("wo", [L, 2, 128, KT, 512], BF16, kind="ExternalInput")
    w1 = nc.dram_tensor("w1", [L, 8, 128, KT, 512], BF16, kind="ExternalInput")
    w2 = nc.dram_tensor("w2", [L, DFF, D], BF16, kind="ExternalInput")
    bqT = nc.dram_tensor("bqT", [L, 128, KT], F32, kind="ExternalInput")
    bkT = nc.dram_tensor("bkT", [L, 128, KT], F32, kind="ExternalInput")
    b1T = nc.dram_tensor("b1T", [L, 128, FT], F32, kind="ExternalInput")
    g1T = nc.dram_tensor("g1T", [L, 128, KT], F32, kind="ExternalInput")
    g2T = nc.dram_tensor("g2T", [L, 128, KT], F32, kind="ExternalInput")
    be2T = nc.dram_tensor("be2T", [L, 128, KT], F32, kind="ExternalInput")
    # K=1 PSUM-seed rows: fc2 gets b2+be1, O-proj gets bo_eff(+be2[l-1])
    b2r = nc.dram_tensor("b2r", [L, 1, D], BF16, kind="ExternalInput")
    bor = nc.dram_tensor("bor", [L, 1, D], BF16, kind="ExternalInput")
    xout = nc.dram_tensor("xout", [D, TOK], F32, kind="ExternalOutput")
    dbg = {}
    if DEBUG:
        for nm, shp, dt in [
            ("dbg_x0", [128, KT, TOK], F32),     # post-embedding x
            ("dbg_ktl", [128, KT, TOK], BF16),   # K proj (own half)
            ("dbg_qt", [128, KT, TOK], BF16),    # Q proj
            ("dbg_attn", [128, KT, TOK], BF16),  # softmax @ V
            ("dbg_r1", [128, KT, TOK], F32),     # post O-drain residual
            ("dbg_ht", [128, FT, TOK], BF16),    # fc1 relu out
            ("dbg_r2", [128, KT, TOK], F32),     # post fc2 residual
            ("dbg_xh2", [128, KT, TOK], F32),    # xhat2 end of layer 0
        ]:
            dbg[nm] = nc.dram_tensor(nm, shp, dt, kind="ExternalOutput")

    from contextlib import ExitStack
    with tile.TileContext(nc) as tc:
        with ExitStack() as _es:
            def _pool(**kw):
                return _es.enter_context(tc.tile_pool(**kw))
            cpool = _pool(name="cpool", bufs=1)
            wp = _pool(name="wp", bufs=2)        # QKVO/fc1 weight chunks
            w2p = _pool(name="w2p", bufs=3)      # fc2 weight row-blocks
            pbig = _pool(name="pbig", bufs=1)    # ht
            ppe = _pool(name="ppe", bufs=1)      # peT (embedding only)
            px = _pool(name="px", bufs=1)        # f32 residual (in-place)
            pxb = _pool(name="pxb", bufs=2)      # bf16 pre-norm rb ping-pong
            pxn = _pool(name="pxn", bufs=2)      # bf16 rstd-scaled rbn
            pat = _pool(name="pat", bufs=1)      # ktl/kto/qt/attn
            pexp = _pool(name="pexp", bufs=2)    # exps per head-pair
            p2 = _pool(name="p2", bufs=2)        # transients (sq/stages)
            prs = _pool(name="prs", bufs=2)      # rstd rows
            pv3 = _pool(name="pv3", bufs=3)      # pavs (3-deep: 2-stage defer)
            bp = _pool(name="bp", bufs=8)        # bias rows
            psd = _pool(name="psd", bufs=1)      # K=1 PSUM-seed rows
            psc = _pool(name="psc", bufs=2, space="PSUM")   # 2-bank tiles
            ppv = _pool(name="ppv", bufs=2, space="PSUM")   # 1-bank tiles
            ppj = _pool(name="ppj", bufs=2, space="PSUM")   # proj psums
            dram = _pool(name="dram", bufs=2, space="DRAM")
            _uid = [0]

            def _nm(tag):
                _uid[0] += 1
                return f"{tag}_{_uid[0]}"

            ident = cpool.tile([128, 128], BF16, tag="ident", name=_nm("ident"))
            make_identity(nc, ident[:])
            onesk = cpool.tile([128, 128], BF16, tag="onesk", name=_nm("onesk"))
            nc.vector.memset(onesk[:], 1.0 / D)
            ones1t = cpool.tile([1, TOK], BF16, tag="ones1t", name=_nm("ones1t"))
            nc.vector.memset(ones1t[:], 1.0)
            maskb_sb = cpool.tile([128, KT], F32, tag="maskb", name=_nm("maskb"))
            nc.sync.dma_start(maskb_sb[:], maskb[:])
            eps_sb = cpool.tile([128, 1], F32, tag="eps", name=_nm("eps"))
            nc.vector.memset(eps_sb[:], EPS)
            # vaug: [tok-part, kt, head*65] fp8; col 64 of each head block is
            # the constant VS used to accumulate the softmax denominator.
            vaug = cpool.tile([128, KT, H * 65], FP8, tag="vaug", name=_nm("vaug"))
            nc.vector.memset(
                vaug[:].rearrange("p t (h x) -> p t h x", x=65)[:, :, :, 64:65], VS)

            def pj():
                return ppj.tile([128, 512], F32, tag="pj", name=_nm("pj"))

            def load_bias8(t, l):
                b = bp.tile([128, KT], F32, tag="bias8", name=_nm("bias8"))
                nc.sync.dma_start(b[:], t[l])
                return b

            def load_seed(t, l):
                s = psd.tile([1, D], BF16, tag="seedrow", name=_nm("seed"))
                nc.sync.dma_start(s[:], t[l])
                return s

            # ---------------- embedding ----------------
            peT_sb = ppe.tile([128, KT, TOK], F32, tag="peT", name=_nm("peT"))
            nc.sync.dma_start(peT_sb[:], peT.rearrange("(t p) n -> p t n", p=128))
            x_cur = px.tile([128, KT, TOK], F32, tag="x", name=_nm("x"))
            for blk in range(TOK // 128):
                idx_t = p2.tile([128, 1], I32, tag="idx", name=_nm("idx"))
                nc.sync.dma_start(idx_t[:], src[blk * 128:(blk + 1) * 128, :])
                gat = p2.tile([128, D], F32, tag="bfs", name=_nm("gat"))
                nc.gpsimd.indirect_dma_start(
                    out=gat[:], out_offset=None, in_=emb[:],
                    in_offset=bass.IndirectOffsetOnAxis(ap=idx_t[:, :1], axis=0),
                )
                gatb = p2.tile([128, D], BF16, tag="lns", name=_nm("gatb"))
                nc.scalar.activation(gatb[:], gat[:], AF.Copy)
                for kt in range(KT):
                    tp = ppj.tile([128, 512], BF16, tag="pj", name=_nm("tp"))
                    nc.tensor.transpose(tp[:, :128], gatb[:, kt * 128:(kt + 1) * 128],
                                        ident[:])
                    nc.vector.scalar_tensor_tensor(
                        out=x_cur[:, kt, blk * 128:(blk + 1) * 128],
                        in0=tp[:, :128], scalar=32.0,
                        in1=peT_sb[:, kt, blk * 128:(blk + 1) * 128],
                        op0=OP.mult, op1=OP.add,
                    )
            rbn_cur = pxn.tile([128, KT, TOK], BF16, tag="xn", name=_nm("xn0"))
            for kk in range(KT):
                nc.scalar.activation(rbn_cur[:, kk, :], x_cur[:, kk, :], AF.Copy)
            if DEBUG:
                nc.sync.dma_start(dbg["dbg_x0"][:], x_cur[:])

            # ------------- layer norm pieces -------------
            def ln_begin():
                st = psc.tile([128, 2, 512], F32, tag="sc", name=_nm("lnst"))
                rb = pxb.tile([128, KT, TOK], BF16, tag="xb", name=_nm("rb"))
                return {"st": st, "rb": rb}

            def ln_stat(s, r, mg):
                nc.scalar.activation(s["rb"][:, mg, :], r[:, mg, :], AF.Copy)
                sq = p2.tile([128, TOK], BF16, tag="sq1", name=_nm("sq"))
                nc.vector.tensor_mul(sq[:], s["rb"][:, mg, :],
                                     s["rb"][:, mg, :])
                nc.tensor.matmul(s["st"][:, 0, :], onesk[:], s["rb"][:, mg, :],
                                 start=(mg == 0), stop=(mg == KT - 1))
                nc.tensor.matmul(s["st"][:, 1, :], onesk[:], sq[:],
                                 start=(mg == 0), stop=(mg == KT - 1))

            def ln_final(s):
                # rstd = sqrt(1/(var+eps)): the approx-reciprocal runs on the
                # raw variance so both the f32 and bf16 sqrt taps come straight
                # off one DVE chain (shorter than sqrt->recip->cast)
                msq = p2.tile([128, TOK], F32, tag="lns", name=_nm("msq"))
                nc.scalar.activation(msq[:], s["st"][:, 0, :], AF.Square)
                inv = p2.tile([128, TOK], F32, tag="lns", name=_nm("inv"))
                nc.vector.scalar_tensor_tensor(
                    out=inv[:], in0=s["st"][:, 1, :], scalar=eps_sb[:, 0:1],
                    in1=msq[:], op0=OP.add, op1=OP.subtract)
                nc.vector.reciprocal_approx_fast(inv[:], inv[:])
                rstd = prs.tile([128, TOK], F32, tag="rstd", name=_nm("rstd"))
                nc.scalar.activation(rstd[:], inv[:], AF.Sqrt)
                rs16 = prs.tile([128, TOK], BF16, tag="rs16", name=_nm("rs16"))
                nc.scalar.activation(rs16[:], inv[:], AF.Sqrt)
                s["rstd"] = rstd
                s["rs16"] = rs16

            def ln_xhat_kk(s, r, kk):
                # in-place normalize of the residual stream (r -> x-hat,
                # WITHOUT the g/be affine - that is folded downstream)
                nc.vector.tensor_sub(r[:, kk, :], r[:, kk, :], s["st"][:, 0, :])
                nc.vector.tensor_mul(r[:, kk, :], r[:, kk, :], s["rstd"][:])

            # ---------------- layers ----------------
            wkh_pre = [None, None]
            for l in range(L):
                bk_sb = load_bias8(bkT, l)
                bq_sb = load_bias8(bqT, l)
                bor_sb = load_seed(bor, l)
                rbn_in = rbn_cur        # rstd-scaled residual (or embedding)

                # --- K projection + chunked pair-AllGather of K^T ---
                # weight loads ride the ACT queue (nc.scalar) one phase ahead
                # of their consumers so the PE never waits on LDWEIGHTS input
                def wload(t, src):
                    w = wp.tile([128, KT, 512], BF16, tag="wproj", name=_nm(t))
                    nc.scalar.dma_start(w[:], src)
                    return w

                # first layer loads its K weights here; later layers get them
                # prefetched during the previous layer's fc2
                wkh_t = wkh_pre if l > 0 else [wload("wk", wk[l, h])
                                               for h in range(2)]
                wvh_t = [None, None]
                wq_tiles = {}
                ktl = pat.tile([128, KT, TOK], BF16, tag="ktl", name=_nm("ktl"))
                kag_in = dram.tile([D, TOK], BF16, tag="kag_in", name=_nm("kag_in"))
                kag_out = [
                    dram.tile([2 * 512, TOK], BF16, tag=f"kag_out{c}",
                              name=_nm("kag_out")) for c in range(2)]
                kto = pat.tile([128, KT, TOK], BF16, tag="kto", name=_nm("kto"))
                for half in range(2):
                    wkh = wkh_t[half]
                    for m in range(4):
                        mg = half * 4 + m
                        pt = pj()
                        for kk in range(KT):
                            nc.tensor.matmul(
                                pt[:], wkh[:, kk, m * 128:(m + 1) * 128],
                                rbn_in[:, kk, :],
                                start=(kk == 0), stop=(kk == KT - 1))
                        nc.scalar.activation(ktl[:, mg, :], pt[:], AF.Identity,
                                             bias=bk_sb[:, mg:mg + 1])
                        nc.sync.dma_start(
                            kag_in[mg * 128:(mg + 1) * 128, :], ktl[:, mg, :])
                    nc.gpsimd.collective_compute(
                        "AllGather", OP.bypass,
                        ins=[kag_in[half * 512:(half + 1) * 512, :]],
                        outs=[kag_out[half][:]],
                        replica_groups=PAIRS,
                    )
                    wvh_t[half] = wload("wv", wv[l, half])
                    # partner-K gathers ride right behind this half's AG
                    for g in range(4 * half, 4 * half + 4):
                        kidx = bp.tile([128, 1], I32, tag="koidx",
                                       name=_nm("koidx"))
                        nc.sync.dma_start(
                            kidx[:], koidx[(g % 4) * 128:(g % 4) * 128 + 128, :])
                        nc.gpsimd.indirect_dma_start(
                            out=kto[:, g, :], out_offset=None,
                            in_=kag_out[g // 4][:],
                            in_offset=bass.IndirectOffsetOnAxis(
                                ap=kidx[:, :1], axis=0),
                        )
                if DEBUG and l == 0:
                    nc.sync.dma_start(dbg["dbg_ktl"][:], ktl[:])

                # --- V projection into vaug (own slots) + fp8 AllGather ---
                # mt-outer so each 128-token row block is complete (both head
                # halves) early; the AG is split in two so partner-V gathers
                # start at the V-phase midpoint instead of after it
                vag_in = dram.tile([TOK, H * 65], FP8, tag="vag_in", name=_nm("vag_in"))
                vag_out = [
                    dram.tile([2 * 256, H * 65], FP8, tag=f"vag_out{c}",
                              name=_nm("vag_out")) for c in range(2)]
                for mt in range(4):
                    for half in range(2):
                        wvh = wvh_t[half]
                        pt = pj()
                        for kk in range(KT):
                            nc.tensor.matmul(
                                pt[:], rbn_in[:, kk, mt * 128:(mt + 1) * 128],
                                wvh[:, kk, :],
                                start=(kk == 0), stop=(kk == KT - 1))
                        vdst = vaug[:, mt, :].rearrange(
                            "p (h x) -> p h x", x=65)[:, half * 8:(half + 1) * 8, 0:64]
                        vsrc = pt[:].rearrange("p (h c) -> p h c", c=64)
                        nc.scalar.activation(vdst, vsrc, AF.Copy, scale=VS)
                    nc.sync.dma_start(
                        vag_in[mt * 128:(mt + 1) * 128, :], vaug[:, mt, :])
                    if mt % 2 == 1:
                        hv = mt // 2
                        nc.gpsimd.collective_compute(
                            "AllGather", OP.bypass,
                            ins=[vag_in[hv * 256:(hv + 1) * 256, :]],
                            outs=[vag_out[hv][:]],
                            replica_groups=PAIRS,
                        )
                        if hv == 0:
                            wq_tiles[0] = wload("wq", wq[l, 0])
                        for mt2 in (2 * hv, 2 * hv + 1):
                            vidx = bp.tile([128, 1], I32, tag="voidx",
                                           name=_nm("voidx"))
                            nc.sync.dma_start(
                                vidx[:], voidx[mt2 * 128:(mt2 + 1) * 128, :])
                            nc.gpsimd.indirect_dma_start(
                                out=vaug[:, 4 + mt2, :], out_offset=None,
                                in_=vag_out[hv][:],
                                in_offset=bass.IndirectOffsetOnAxis(
                                    ap=vidx[:, :1], axis=0),
                            )
                wq_tiles[1] = wload("wq", wq[l, 1])

                # --- Q projection woven with attention at 2-MM granularity ---
                qt = pat.tile([128, KT, TOK], BF16, tag="qt", name=_nm("qt"))
                # per-m-tile attn tiles: O-proj's kk-th matmul depends only on
                # tile kk, so the tail heads' softmax chains overlap O-proj
                attn_t = [pat.tile([128, TOK], BF16, tag=f"attn{j}",
                                   name=_nm("attn")) for j in range(KT)]
                recips_d = dram.tile([H, TOK], F32, tag="recips_d", name=_nm("recd"))
                recips_r = dram.tile([H, TOK], F32, tag="recips_r", name=_nm("recr"))
                exps_tiles = {}

                def kt_lhs(kt, mj, prow):
                    if kt < 4:
                        return ktl[prow:prow + 64, mj, kt * 128:(kt + 1) * 128]
                    return kto[prow:prow + 64, mj, (kt - 4) * 128:(kt - 3) * 128]

                def s_unit(mj, kt):
                    # one kt-block of scores for BOTH heads of m-tile mj; a
                    # single exp instruction covers the pair (same mask col)
                    if kt == 0:
                        exps_tiles[mj] = pexp.tile([128, KT, 2, TOK], FP8,
                                                   tag="exps", name=_nm("ex"))
                    ex = exps_tiles[mj]
                    st = psc.tile([128, 2, 512], F32, tag="sc", name=_nm("sc"))
                    for j in range(2):
                        prow = j * 64
                        nc.tensor.matmul(
                            st[:, j, :], kt_lhs(kt, mj, prow),
                            qt[prow:prow + 64, mj, :],
                            start=True, stop=True)
                    nc.scalar.activation(
                        ex[:, kt, :, :], st[:, :, :], AF.Exp,
                        scale=DK ** -0.5, bias=maskb_sb[:, kt:kt + 1])

                av_state = {}
                av_p1 = {}
                av_p2 = {}

                def av_stage1(h):
                    # one head-slot after the dT reload was issued: the recip
                    # never head-of-line-blocks the DVE queue on that DMA
                    mj, pavs, dT = av_p1.pop(h)
                    nc.vector.reciprocal_approx_fast(dT[:], dT[:])
                    nc.gpsimd.dma_start(
                        recips_r[h:h + 1, :].rearrange(
                            "o (p f) -> (o p) f", p=128), dT[:])
                    rbc = p2.tile([64, TOK], F32, tag="lns", name=_nm("rbc"))
                    nc.sync.dma_start(
                        rbc[:],
                        recips_r[h:h + 1, None, :].to_broadcast((1, 64, TOK)))
                    av_p2[h] = (mj, pavs, rbc)

                def av_stage2(h):
                    # and the attn mul one further slot later, once rbc landed
                    mj, pavs, rbc = av_p2.pop(h)
                    if h % 2 == 0:
                        nc.vector.tensor_mul(
                            attn_t[mj][0:64, :], pavs[0:64, :], rbc[:])
                    else:
                        stg = p2.tile([64, TOK], BF16, tag="stage",
                                      name=_nm("stg"))
                        nc.vector.tensor_mul(stg[:], pavs[0:64, :], rbc[:])
                        nc.sync.dma_start(attn_t[mj][64:128, :], stg[:])

                def av_unit(h, c):
                    mj = h // 2
                    ex = exps_tiles[mj]
                    if c == 0:
                        av_state[h] = ppv.tile([128, 512], F32, tag="pav",
                                               name=_nm("pav"))
                    pav = av_state[h]
                    vo = 65 * h
                    hs = h % 2
                    for p in (2 * c, 2 * c + 1):
                        nc.tensor.matmul(
                            pav[0:65, :], vaug[:, 2 * p:2 * p + 2, vo:vo + 65],
                            ex[:, 2 * p:2 * p + 2, hs, :],
                            start=(p == 0), stop=(p == 3),
                            perf_mode=DR)
                    if c == 1:
                        if hs == 1:
                            exps_tiles.pop(mj)
                        av_state.pop(h)
                        # free the PSUM bank fast; the rest of the softmax
                        # chain lags 1-2 head-slots behind the SBUF copy
                        pavs = pv3.tile([65, TOK], F32, tag="pavs", name=_nm("pavs"))
                        nc.vector.tensor_copy(pavs[:], pav[0:65, :])
                        nc.sync.dma_start(recips_d[h:h + 1, :], pavs[64:65, :])
                        # approx reciprocal is only valid on full-partition
                        # tiles: round-trip the denom row through DRAM as
                        # [128, 4] (130ns) instead of a 3.3us 1-lane recip
                        dT = bp.tile([128, 4], F32, tag="dT", name=_nm("dT"))
                        nc.gpsimd.dma_start(
                            dT[:], recips_d[h:h + 1, :].rearrange(
                                "o (p f) -> (o p) f", p=128))
                        av_p1[h] = (mj, pavs, dT)
                        if (h - 1) in av_p1:
                            av_stage1(h - 1)
                        if (h - 2) in av_p2:
                            av_stage2(h - 2)

                qpt = {}

                def q_unit(mg, c):
                    if c == 0:
                        qpt[mg] = pj()
                    wqh = wq_tiles[mg // 4]
                    m = mg % 4
                    for kk in (2 * c, 2 * c + 1):
                        nc.tensor.matmul(
                            qpt[mg][:], wqh[:, kk, m * 128:(m + 1) * 128],
                            rbn_in[:, kk, :],
                            start=(kk == 0), stop=(kk == KT - 1))
                    if c == 3:
                        nc.vector.tensor_scalar_add(
                            qt[:, mg, :], qpt.pop(mg)[:], bq_sb[:, mg:mg + 1])

                woh_t = [None, None]
                for c in range(4):
                    q_unit(0, c)
                for mg in range(KT):
                    if mg == 5:
                        woh_t[0] = wload("wo", wo[l, 0])
                    if mg == 7:
                        woh_t[1] = wload("wo", wo[l, 1])
                    units = []
                    if mg >= 1:
                        h0 = 2 * (mg - 1)
                        units += [("av", h0, 0), ("av", h0, 1),
                                  ("av", h0 + 1, 0), ("av", h0 + 1, 1)]
                    if mg + 1 < KT:
                        units += [("q", mg + 1, c) for c in range(4)]
                    s_units = [("s", mg, kt) for kt in range(KT)]
                    woven = []
                    oi = 0
                    for su in s_units:
                        woven.append(su)
                        if oi < len(units):
                            woven.append(units[oi]); oi += 1
                    woven += units[oi:]
                    for u in woven:
                        if u[0] == "s":
                            s_unit(u[1], u[2])
                        elif u[0] == "q":
                            q_unit(u[1], u[2])
                        else:
                            av_unit(u[1], u[2])
                for h in (14, 15):
                    for c in range(2):
                        av_unit(h, c)
                # head 15 is the attention tail: skip the DRAM broadcast
                # round-trip and use an exact in-SBUF reciprocal plus a K=1
                # PE outer-product broadcast (the PE is idle here anyway)
                mj15, pavs15, _dT15 = av_p1.pop(15)
                nc.vector.reciprocal(pavs15[64:65, :], pavs15[64:65, :])
                rrow = p2.tile([1, TOK], BF16, tag="stage", name=_nm("rrow"))
                nc.scalar.activation(rrow[0:1, :], pavs15[64:65, :], AF.Copy)
                rbc_ps = pj()
                nc.tensor.matmul(rbc_ps[0:64, :], ones1t[0:1, 0:64],
                                 rrow[0:1, :], start=True, stop=True)
                av_stage2(14)
                stg15 = p2.tile([64, TOK], BF16, tag="stage", name=_nm("stg15"))
                nc.vector.tensor_mul(stg15[:], pavs15[0:64, :], rbc_ps[0:64, :])
                nc.sync.dma_start(attn_t[mj15][64:128, :], stg15[:])
                if DEBUG and l == 0:
                    nc.sync.dma_start(dbg["dbg_qt"][:], qt[:])
                    for j in range(KT):
                        nc.sync.dma_start(dbg["dbg_attn"][:, j, :], attn_t[j][:])

                # --- O projection + residual; LN1 stats woven per m-tile ---
                # PSUM is seeded with bo_eff (+be2[l-1]); for l>=1 the drain
                # applies g2[l-1] to the x-hat residual stream.
                g1_sb = load_bias8(g1T, l)
                g2p_sb = load_bias8(g2T, l - 1) if l >= 1 else None
                # kk-outer over 4 accumulators, two passes (mg 0-3 on psc with
                # woh0, mg 4-7 on ppv+ppj with woh1): the attention tail's
                # softmax chains overlap the O matmuls, and woh0 dies at the
                # pass boundary so the first fc1 weight chunk loads early
                o_a = psc.tile([128, 2, 512], F32, tag="sc", name=_nm("oa"))
                o_a2 = psc.tile([128, 2, 512], F32, tag="sc", name=_nm("oa2"))
                o_b = ppv.tile([128, 512], F32, tag="pav", name=_nm("ob"))
                o_c = ppv.tile([128, 512], F32, tag="pav", name=_nm("oc"))
                o_d = pj()
                o_e = pj()
                ops = [o_a[:, 0, :], o_a[:, 1, :], o_a2[:, 0, :], o_a2[:, 1, :],
                       o_b[:], o_c[:], o_d[:], o_e[:]]
                for half in range(2):
                    mgs = range(4 * half, 4 * half + 4)
                    for mg in mgs:
                        nc.tensor.matmul(
                            ops[mg], bor_sb[0:1, mg * 128:(mg + 1) * 128],
                            ones1t[0:1, :], start=True, stop=False)
                    for kk in range(KT):
                        for mg in mgs:
                            nc.tensor.matmul(
                                ops[mg],
                                woh_t[half][:, kk, (mg % 4) * 128:(mg % 4 + 1) * 128],
                                attn_t[kk][:],
                                start=False, stop=(kk == KT - 1))
                    if half == 0:
                        w1e_next = [wload("w1e", w1[l, 0])]

                def o_drain(mg):
                    if l == 0:
                        nc.vector.tensor_add(
                            x_cur[:, mg, :], ops[mg], x_cur[:, mg, :])
                    else:
                        nc.vector.scalar_tensor_tensor(
                            out=x_cur[:, mg, :], in0=x_cur[:, mg, :],
                            scalar=g2p_sb[:, mg:mg + 1], in1=ops[mg],
                            op0=OP.mult, op1=OP.add)

                # drains 0-1 must precede ln_begin (its stats tile takes o_a's
                # PSUM banks); the rest interleave with the stats
                o_drain(0)
                o_drain(1)
                ln1 = ln_begin()
                for mg in range(2, KT):
                    o_drain(mg)
                    ln_stat(ln1, x_cur, mg - 2)
                ln_stat(ln1, x_cur, 6)
                ln_stat(ln1, x_cur, 7)
                ln_final(ln1)
                if DEBUG and l == 0:
                    nc.sync.dma_start(dbg["dbg_r1"][:], x_cur[:])
                rb1 = ln1["rb"]
                # rbn1 = rb1 * rstd1 : fc1's folded weights consume this
                rbn1 = pxn.tile([128, KT, TOK], BF16, tag="xn", name=_nm("rbn1"))
                for kk in range(KT):
                    nc.vector.tensor_mul(rbn1[:, kk, :], rb1[:, kk, :],
                                         ln1["rs16"][:])

                # --- fc1 on the rstd-scaled residual (folded weights) ---
                b1_sb = bp.tile([128, FT], F32, tag="bias32", name=_nm("b1"))
                nc.sync.dma_start(b1_sb[:], b1T[l])
                b2r_sb = load_seed(b2r, l)
                g2_sb = load_bias8(g2T, l)
                ht = pbig.tile([128, FT, TOK], BF16, tag="big32", name=_nm("ht"))
                for e in range(8):
                    w1e = w1e_next[0]
                    if e < 7:
                        w1e_next[0] = wload("w1e", w1[l, e + 1])
                    for m in range(4):
                        fm = e * 4 + m
                        # 4-deep psum rotation (pj + ppv) keeps the PE fed
                        if fm % 2 == 0:
                            pt = pj()
                        else:
                            pt = ppv.tile([128, 512], F32, tag="pav",
                                          name=_nm("fpv"))
                        for kk in range(KT):
                            nc.tensor.matmul(
                                pt[:], w1e[:, kk, m * 128:(m + 1) * 128],
                                rbn1[:, kk, :],
                                start=(kk == 0), stop=(kk == KT - 1))
                        nc.scalar.activation(ht[:, fm, :], pt[:], AF.Relu,
                                             bias=b1_sb[:, fm:fm + 1])
                if DEBUG and l == 0:
                    nc.sync.dma_start(dbg["dbg_ht"][:], ht[:])
                # normalize the residual in place (r1 -> xhat1; g1/be1 are
                # folded into the fc2 drain/seed)
                for kk in range(KT):
                    ln_xhat_kk(ln1, x_cur, kk)

                # --- fc2 kk-outer over 8 accumulators; contiguous w2 loads ---
                f_a = psc.tile([128, 2, 512], F32, tag="sc", name=_nm("fa"))
                f_a2 = psc.tile([128, 2, 512], F32, tag="sc", name=_nm("fa2"))
                f_b = ppv.tile([128, 512], F32, tag="pav", name=_nm("fb"))
                f_c = ppv.tile([128, 512], F32, tag="pav", name=_nm("fc"))
                f_d = pj()
                f_e = pj()
                fps = [f_a[:, 0, :], f_a[:, 1, :], f_a2[:, 0, :], f_a2[:, 1, :],
                       f_b[:], f_c[:], f_d[:], f_e[:]]
                for mg in range(KT):
                    nc.tensor.matmul(
                        fps[mg], b2r_sb[0:1, mg * 128:(mg + 1) * 128],
                        ones1t[0:1, :], start=True, stop=False)

                # w2 row-blocks: split each load across both HWDGE queues and
                # run the prefetch two chunks deep so the PE never starves
                def w2load(kk):
                    t = w2p.tile([128, 1024], BF16, tag="w2c", name=_nm("w2c"))
                    nc.sync.dma_start(
                        t[:, 0:512], w2[l, kk * 128:(kk + 1) * 128, 0:512])
                    nc.scalar.dma_start(
                        t[:, 512:1024], w2[l, kk * 128:(kk + 1) * 128, 512:1024])
                    return t

                w2q = [w2load(0), w2load(1)]
                for kk in range(FT):
                    if kk == 20 and l + 1 < L:
                        wkh_pre = [wload("wk", wk[l + 1, h]) for h in range(2)]
                    w2c = w2q.pop(0)
                    if kk < FT - 2:
                        w2q.append(w2load(kk + 2))
                    for mg in range(KT):
                        nc.tensor.matmul(
                            fps[mg], w2c[:, mg * 128:(mg + 1) * 128],
                            ht[:, kk, :],
                            start=False, stop=(kk == FT - 1))
                def f_drain(mg):
                    # r2 = xhat1*g1 + (h@w2 + b2 + be1)   (seeded PSUM)
                    nc.vector.scalar_tensor_tensor(
                        out=x_cur[:, mg, :], in0=x_cur[:, mg, :],
                        scalar=g1_sb[:, mg:mg + 1], in1=fps[mg],
                        op0=OP.mult, op1=OP.add)

                # drains 0-1 free f_a's banks before ln_begin claims them;
                # the remaining drains interleave with the stats
                f_drain(0)
                f_drain(1)
                ln2 = ln_begin()
                for mg in range(2, KT):
                    f_drain(mg)
                    ln_stat(ln2, x_cur, mg - 2)
                ln_stat(ln2, x_cur, 6)
                ln_stat(ln2, x_cur, 7)
                if DEBUG and l == 0:
                    nc.sync.dma_start(dbg["dbg_r2"][:], x_cur[:])
                ln_final(ln2)
                rb2 = ln2["rb"]
                if l < L - 1:
                    rbn_cur = pxn.tile([128, KT, TOK], BF16, tag="xn",
                                       name=_nm("rbn2"))
                    for kk in range(KT):
                        nc.vector.tensor_mul(rbn_cur[:, kk, :], rb2[:, kk, :],
                                             ln2["rs16"][:])
                    for kk in range(KT):
                        ln_xhat_kk(ln2, x_cur, kk)
                    if DEBUG and l == 0:
                        nc.sync.dma_start(dbg["dbg_xh2"][:], x_cur[:])
                else:
                    be2_sb = load_bias8(be2T, l)
                    for kk in range(KT):
                        ln_xhat_kk(ln2, x_cur, kk)
                        nc.scalar.activation(
                            x_cur[:, kk, :], x_cur[:, kk, :], AF.Identity,
                            bias=be2_sb[:, kk:kk + 1], scale=g2_sb[:, kk:kk + 1])

            nc.sync.dma_start(
                xout.rearrange("(t p) n -> p t n", p=128), x_cur[:])

    return nc


MAXW = 1


def split_wait_overflow(nc, maxw=MAXW):
    """walrus in this toolchain rejects instructions with more than one sem
    wait; split excess waits onto preceding NoOp carriers on the same engine."""
    for f in nc.m.functions:
        for bb in f.blocks:
            if not any(i.sync_info and len(i.sync_info.on_wait) > maxw
                       for i in bb.instructions):
                continue
            newlist = []
            for inst in bb.instructions:
                si = inst.sync_info
                if si and len(si.on_wait) > maxw:
                    waits = list(si.on_wait)
                    extra, keep = waits[:-maxw], waits[-maxw:]
                    for i in range(0, len(extra), maxw):
                        newlist.append(mybir.InstNoOp(
                            name=f"{inst.name}-ws{i}", opcode="NoOp",
                            engine=inst.engine, debug=inst.debug, ins=[], outs=[],
                            sync_info=mybir.SyncInfo(
                                on_wait=extra[i:i + maxw], on_update=[]),
                        ))
                    inst.sync_info = mybir.SyncInfo(
                        on_wait=keep, on_update=list(si.on_update))
                newlist.append(inst)
            bb.instructions = newlist


def _get_nc():
    global _NC
    if _NC is None:
        _NC = _build_nc()
        # populate .instr bytes for extended InstISA (custom DVE ops);
        # raw Bass skips this codegen pass
        mybir.codegen_inst_isa_subclasses(_NC)
        split_wait_overflow(_NC)
    return _NC


def _to_bf16(a):
    return np.asarray(a, dtype=np.float32).astype(ml_dtypes.bfloat16)


def _bias_t(v, kt=KT):
    # [L, d] -> [L, 128, d//128] with column t = v[:, 128t:128t+128]
    v = np.asarray(v, dtype=np.float32)
    return np.ascontiguousarray(v.reshape(L, kt, 128).transpose(0, 2, 1))


def _fold(w, g, be):
    """LayerNorm fold: returns (w'', bias_delta) with
    x_hat @ w = (rstd*r) @ w'' + bias_delta  (per token rstd)."""
    wp = g[:, None] * w
    c = wp.sum(axis=0)
    return wp - c[None, :] / D, be @ w


def _tile_w(w, nchunk):
    """[L, D, n*512] -> [L, n, 128, KT, 512] matching the kernel's SBUF
    weight-tile layout so the DMA reads are contiguous."""
    Lw, Din, Dout = w.shape
    out = w.reshape(Lw, KT, 128, nchunk, 512).transpose(0, 3, 2, 1, 4)
    return np.ascontiguousarray(out)


def kernel(**inputs):
    nc = _get_nc()

    src = np.asarray(inputs["src"]).astype(np.int32).reshape(-1)      # [4096]
    src_mask = np.asarray(inputs["src_mask"]).astype(np.float32)      # [B,1,1,S]
    emb = np.asarray(inputs["emb"], dtype=np.float32)
    pe = np.asarray(inputs["pe"], dtype=np.float32)

    wq_f = np.asarray(inputs["wq"], dtype=np.float32).copy()
    wk_f = np.asarray(inputs["wk"], dtype=np.float32).copy()
    wv_f = np.asarray(inputs["wv"], dtype=np.float32).copy()
    wo_f = np.asarray(inputs["wo"], dtype=np.float32)
    w1_f = np.asarray(inputs["w1"], dtype=np.float32).copy()
    w2_f = np.asarray(inputs["w2"], dtype=np.float32)
    bq_f = np.asarray(inputs["bq"], dtype=np.float32).copy()
    bk_f = np.asarray(inputs["bk"], dtype=np.float32).copy()
    bv_f = np.asarray(inputs["bv"], dtype=np.float32).copy()
    bo_f = np.asarray(inputs["bo"], dtype=np.float32)
    b1_f = np.asarray(inputs["b1"], dtype=np.float32).copy()
    b2_f = np.asarray(inputs["b2"], dtype=np.float32)
    g1_f = np.asarray(inputs["g1"], dtype=np.float32)
    be1_f = np.asarray(inputs["be1"], dtype=np.float32)
    g2_f = np.asarray(inputs["g2"], dtype=np.float32)
    be2_f = np.asarray(inputs["be2"], dtype=np.float32)

    # fold LN1 into fc1 (all layers); fold LN2[l-1] into QKV[l] (l >= 1)
    for l in range(L):
        w1_f[l], d1 = _fold(w1_f[l], g1_f[l], be1_f[l])
        b1_f[l] = b1_f[l] + d1
        if l >= 1:
            g, be = g2_f[l - 1], be2_f[l - 1]
            wq_f[l], dq = _fold(wq_f[l], g, be)
            bq_f[l] = bq_f[l] + dq
            wk_f[l], dk_ = _fold(wk_f[l], g, be)
            bk_f[l] = bk_f[l] + dk_
            wv_f[l], dv = _fold(wv_f[l], g, be)
            bv_f[l] = bv_f[l] + dv
    # fold the V bias through the O projection: attn rows sum to 1, so
    # out = attn@(V + bv) @ wo + bo = attn@V@wo + (bv@wo + bo)
    bo_eff = np.stack([bo_f[l] + bv_f[l] @ wo_f[l] for l in range(L)])
    # O-proj PSUM seed: bo_eff plus the previous layer's LN2 shift (be2)
    bor_np = bo_eff.copy()
    for l in range(1, L):
        bor_np[l] = bor_np[l] + be2_f[l - 1]
    # fc2 PSUM seed: b2 plus this layer's LN1 shift (be1)
    b2r_np = b2_f + be1_f

    shared = {
        "emb": emb,
        "wq": _tile_w(_to_bf16(wq_f), 2), "wk": _tile_w(_to_bf16(wk_f), 2),
        "wv": _tile_w(_to_bf16(wv_f), 2), "wo": _tile_w(_to_bf16(wo_f), 2),
        "w1": _tile_w(_to_bf16(w1_f), 8), "w2": _to_bf16(w2_f),
        "bqT": _bias_t(bq_f), "bkT": _bias_t(bk_f),
        "b1T": _bias_t(b1_f, FT),
        "g1T": _bias_t(g1_f),
        "g2T": _bias_t(g2_f), "be2T": _bias_t(be2_f),
        "b2r": np.ascontiguousarray(_to_bf16(b2r_np).reshape(L, 1, D)),
        "bor": np.ascontiguousarray(_to_bf16(bor_np).reshape(L, 1, D)),
    }

    in_maps = []
    for c in range(NCORES):
        b = c // 2
        half = c % 2
        m = dict(shared)
        m["src"] = np.ascontiguousarray(
            src[c * TOK:(c + 1) * TOK].reshape(TOK, 1))
        m["peT"] = np.ascontiguousarray(
            pe[half * TOK:half * TOK + TOK, :D].T.astype(np.float32))
        mb = (src_mask[b, 0, 0, :] - 1.0) * 1e9
        own = slice(half * TOK, half * TOK + TOK)
        pair = slice((1 - half) * TOK, (1 - half) * TOK + TOK)
        mb_perm = np.concatenate([mb[own], mb[pair]])
        m["maskb"] = np.ascontiguousarray(
            mb_perm.reshape(KT, 128).T.astype(np.float32))
        o = 1 - half  # pair-local rank of the partner
        m["koidx"] = np.ascontiguousarray(
            (np.arange(512, dtype=np.int32) + o * 512).reshape(512, 1))
        # split V-AG layout: vag_out[hv] holds [rank0 rows, rank1 rows] of
        # 256-token slabs; partner token (mt*128+p) sits at o*256+(mt%2)*128+p
        vo = np.empty(TOK, dtype=np.int32)
        ar = np.arange(128, dtype=np.int32)
        for mt in range(4):
            vo[mt * 128:(mt + 1) * 128] = o * 256 + (mt % 2) * 128 + ar
        m["voidx"] = np.ascontiguousarray(vo.reshape(TOK, 1))
        in_maps.append(m)

    res = run_bass_kernel_spmd(nc, in_maps, list(range(NCORES)))
    out = np.empty((B * S, D), dtype=np.float32)
    for c in range(NCORES):
        out[c * TOK:(c + 1) * TOK] = res.results[c]["xout"].T
    return out.reshape(B, S, D)
